# revision 1
# baseline (speedup 1.0000x reference)
"""Trainium2 Bass kernel for nn_Expert_layer2 (dense per-sample HWxHW attention block).

Sharding: 8 cores = 4 samples x 2 query-halves. Each core receives its sample's
inputs in *band order* (band 0 = the core's query-half rows, band 1 = the other
half), computes the conv/GN/LN pre-projections and the attention for its 2048
queries over all 4096 keys, pair-AllGathers the normalized attention output,
then runs the conv head redundantly in global row order. Host takes pred/xmin
from the even core of each pair.
"""

from contextlib import ExitStack

import numpy as np
import concourse.bass as bass
from concourse import bacc
import concourse.tile as tile
import concourse.mybir as mybir
import concourse.bass_isa as bass_isa
from concourse.bass_utils import run_bass_kernel_spmd

F32 = mybir.dt.float32
F32R = mybir.dt.float32r
AF = mybir.ActivationFunctionType
ALU = mybir.AluOpType

B, C1, K, H, W = 4, 64, 80, 64, 64
HW = H * W
HALF = HW // 2
EPS = 1e-5
GATE_SCALE = 0.1
ISQK = float(1.0 / np.sqrt(np.float32(K)))

_BUILT = None


def _build():
    nc = bacc.Bacc("TRN2", target_bir_lowering=False, num_devices=8)

    dd = {}
    dd["x1b"] = nc.dram_tensor("x1b", [128, 2, 34, 66], F32, kind="ExternalInput")
    dd["x2b"] = nc.dram_tensor("x2b", [128, 2, 34, 66], F32, kind="ExternalInput")
    dd["wpack"] = nc.dram_tensor("wpack", [128, 2471], F32, kind="ExternalInput")
    dd["onesrow"] = nc.dram_tensor("onesrow", [4096], F32, kind="ExternalInput")
    dd["pred_o"] = nc.dram_tensor("pred", [4096], F32, kind="ExternalOutput")
    dd["xmin_o"] = nc.dram_tensor("xminv", [4096], F32, kind="ExternalOutput")
    dd["cc_in"] = nc.dram_tensor("cc_in", [4, 80, 512], F32)
    dd["cc_out"] = nc.dram_tensor("cc_out", [4, 2, 80, 512], F32)

    with tile.TileContext(nc) as tc:
        _body(nc, tc, dd)
    nc.finalize()
    return nc


def _body(nc, tc, dd):
    onesrow = dd["onesrow"]
    ctx = ExitStack()
    with ctx:
        pw = ctx.enter_context(tc.tile_pool(name="pw", bufs=1))
        prow = ctx.enter_context(tc.tile_pool(name="prow", bufs=1))
        pscr = ctx.enter_context(tc.tile_pool(name="pscr", bufs=2))
        pror = ctx.enter_context(tc.tile_pool(name="pror", bufs=1))
        # PSUM: one pool; tags: pstat(x2) cps(x2) scps(x2) pvps(x2) = 8 banks
        psm = ctx.enter_context(tc.tile_pool(name="psm", bufs=2, space="PSUM"))

        # ------------ persistent weights (one packed DMA) ------------
        wpk = pw.tile([128, 2471], F32R, tag="wpk")
        nc.sync.dma_start(out=wpk[:, 0:720], in_=dd["wpack"][:, 0:720].bitcast(F32R))
        nc.gpsimd.dma_start(out=wpk[:, 720:1800], in_=dd["wpack"][:, 720:1800].bitcast(F32R))
        nc.sync.dma_start(out=wpk[:, 1800:2471], in_=dd["wpack"][:, 1800:2471].bitcast(F32R))
        off = [0]

        def _wslice(n):
            a = wpk[:, off[0]:off[0] + n]
            off[0] += n
            return a

        wt1 = _wslice(720).rearrange("p (t f) -> p t f", t=9)
        wtg = _wslice(360).rearrange("p (t f) -> p t f", t=9)
        wth = _wslice(720).rearrange("p (t f) -> p t f", t=9)
        ws = {}
        for nm, fd in (("wid", 80), ("wg2", 80), ("wh2", 1), ("lhsq", 80), ("lhsk", 80),
                       ("lhsv", 80), ("indic", 5), ("indicT", 80), ("indicg", 5),
                       ("indicgT", 40)):
            ws[nm] = _wslice(fd)
        vec_sb = _wslice(12).bitcast(F32)
        ident_sb = _wslice(128).bitcast(F32)
        assert off[0] == 2471
        ones_col = pw.tile([128, 1], F32R, tag="onescol")
        nc.sync.dma_start(out=ones_col[:], in_=onesrow[0:128, None].bitcast(F32R))
        eps_col = pw.tile([128, 1], F32, tag="epscol")
        nc.vector.memset(eps_col[:], EPS)

        gn1_g, gn1_b = vec_sb[0:80, 0:1], vec_sb[0:80, 1:2]
        gnid_g, gnid_b = vec_sb[0:80, 2:3], vec_sb[0:80, 3:4]
        gng_g, gng_b = vec_sb[0:40, 4:5], vec_sb[0:40, 5:6]
        gnh_g, gnh_b = vec_sb[0:80, 6:7], vec_sb[0:80, 7:8]
        lno_g, lno_b = vec_sb[0:80, 8:9], vec_sb[0:80, 9:10]
        bg2_ap = vec_sb[0:80, 10:11]
        bh2_ap = vec_sb[0:1, 11:12]

        # ============ helpers ============
        def token_stats(nm, ntok, chunk_fn, neg_mr, mr_target_ap):
            """chunk_fn(c) -> f32r AP [128, 8, 64] of 512 tokens (pad rows zero).
            Returns (r_row, mr_row) SBUF [1, ntok] tiles (mr_row None if
            mr_target_ap given, in which case the r*m row is DMA'd there)."""
            nchunk = ntok // 512
            dn = nc.dram_tensor(f"dn_{nm}", [2, ntok], F32)
            up = nc.dram_tensor(f"up_{nm}", [2, ntok], F32)
            for c in range(nchunk):
                src = chunk_fn(c)
                sqs = pscr.tile([1, 1024], F32, tag="csq")
                ps_s = psm.tile([1, 512], F32, tag="pstat")
                nc.tensor.matmul(ps_s[:], ones_col[:], src, start=True, stop=True)
                nc.vector.tensor_copy(sqs[:, 0:512], ps_s[:])
                sq = pscr.tile([128, 8, 64], F32R, tag="stq")
                nc.scalar.activation(sq[:], src.bitcast(F32), AF.Square)
                ps_q = psm.tile([1, 512], F32, tag="pstat")
                nc.tensor.matmul(ps_q[:], ones_col[:], sq[:], start=True, stop=True)
                nc.scalar.copy(sqs[:, 512:1024], ps_q[:])
                nc.sync.dma_start(out=dn[:, c * 512:(c + 1) * 512], in_=sqs[:])
            ncol = ntok // 128
            st = prow.tile([128, 2, ncol], F32, tag=f"stt_{nm}")
            nc.sync.dma_start(out=st[:], in_=dn.rearrange("r (p c) -> p r c", p=128))
            m = prow.tile([128, ncol], F32, tag=f"m_{nm}")
            nc.vector.tensor_scalar(m[:], st[:, 0, :], 1.0 / K, None, ALU.mult)
            var = prow.tile([128, ncol], F32, tag=f"v_{nm}")
            nc.vector.tensor_scalar(var[:], st[:, 1, :], 1.0 / K, None, ALU.mult)
            msq = prow.tile([128, ncol], F32, tag=f"ms_{nm}")
            nc.vector.tensor_tensor(msq[:], m[:], m[:], ALU.mult)
            nc.vector.tensor_tensor(var[:], var[:], msq[:], ALU.subtract)
            nc.scalar.activation(var[:], var[:], AF.Sqrt, bias=eps_col[:])
            rup = prow.tile([128, 2, ncol], F32, tag=f"ru_{nm}")
            nc.vector.reciprocal(rup[:, 0, :], var[:])
            nc.vector.tensor_tensor(rup[:, 1, :], rup[:, 0, :], m[:], ALU.mult)
            if neg_mr:
                nc.vector.tensor_scalar(rup[:, 1, :], rup[:, 1, :], -1.0, None, ALU.mult)
            nc.sync.dma_start(out=up.rearrange("r (p c) -> p r c", p=128), in_=rup[:])
            r_row = pror.tile([1, 4096], F32, tag="rrow", name="r_row")[:, 0:ntok]
            nc.sync.dma_start(out=r_row, in_=up[0, None, :])
            if mr_target_ap is not None:
                nc.sync.dma_start(out=mr_target_ap, in_=up[1, None, :].bitcast(F32R))
                return r_row, None
            mr_row = pror.tile([1, 4096], F32, tag="rrow", name="mr_row")[:, 0:ntok]
            nc.sync.dma_start(out=mr_row, in_=up[1, None, :])
            return r_row, mr_row

        def conv3x3(wt_sb, nout, win_fn, out_sb, acc, accsq):
            """win_fn(c8, dy, dx) -> rhs AP [128, 8, 64]. out_sb: [128, 4096] f32r."""
            for c8 in range(8):
                ps = psm.tile([nout, 512], F32, tag="cps")
                for tap in range(9):
                    dy, dx = divmod(tap, 3)
                    nc.tensor.matmul(ps[:], wt_sb[:, tap, :], win_fn(c8, dy, dx),
                                     start=(tap == 0), stop=(tap == 8))
                nc.scalar.activation(out_sb[0:nout, c8 * 512:(c8 + 1) * 512], ps[:],
                                     AF.Copy, accum_out=acc[:, c8:c8 + 1])
                sq = pscr.tile([128, 512], F32, tag="csq")
                nc.scalar.activation(sq[0:nout, :], ps[:], AF.Square,
                                     accum_out=accsq[:, c8:c8 + 1])

        def gn_params(nm, acc, accsq, nch, ind_sb, indT_sb, gamma, beta, inv_n):
            s_ch = prow.tile([nch, 2], F32, tag=f"sch_{nm}")
            nc.vector.tensor_reduce(s_ch[:, 0:1], acc[:], mybir.AxisListType.X, ALU.add)
            nc.vector.tensor_reduce(s_ch[:, 1:2], accsq[:], mybir.AxisListType.X, ALU.add)
            chs = pscr.tile([128, 2], F32R, tag="chs")
            nc.vector.memset(chs[:].bitcast(F32), 0.0)
            nc.vector.tensor_copy(chs[0:nch, :], s_ch[:])
            ps_g = psm.tile([5, 2], F32, tag="pstat")
            nc.tensor.matmul(ps_g[:], ind_sb, chs[:], start=True, stop=True)
            gst = pscr.tile([128, 2], F32R, tag="gst")
            nc.vector.memset(gst[:].bitcast(F32), 0.0)
            nc.vector.tensor_scalar(gst[0:5, 0:1], ps_g[:, 0:1], inv_n, None, ALU.mult)
            e2 = prow.tile([5, 1], F32, tag=f"e2_{nm}")
            nc.vector.tensor_scalar(e2[:], ps_g[:, 1:2], inv_n, None, ALU.mult)
            vr = prow.tile([5, 1], F32, tag=f"vr_{nm}")
            nc.vector.tensor_tensor(vr[:], gst[0:5, 0:1].bitcast(F32),
                                    gst[0:5, 0:1].bitcast(F32), ALU.mult)
            nc.vector.tensor_tensor(vr[:], e2[:], vr[:], ALU.subtract)
            nc.scalar.activation(vr[:], vr[:], AF.Sqrt, bias=eps_col[0:5, :])
            rst = prow.tile([5, 1], F32, tag=f"rst_{nm}")
            nc.vector.reciprocal(rst[:], vr[:])
            nc.vector.tensor_copy(gst[0:5, 1:2], rst[:])
            ps_bc = psm.tile([nch, 2], F32, tag="pstat")
            nc.tensor.matmul(ps_bc[:], indT_sb, gst[:], start=True, stop=True)
            a = prow.tile([nch, 1], F32, tag=f"a_{nm}")
            c = prow.tile([nch, 1], F32, tag=f"c_{nm}")
            nc.vector.tensor_tensor(a[:], gamma, ps_bc[:, 1:2], ALU.mult)
            nc.vector.tensor_tensor(c[:], ps_bc[:, 0:1], a[:], ALU.mult)
            nc.vector.tensor_tensor(c[:], beta, c[:], ALU.subtract)
            return a, c

        # ============ stage 1a: x1 conv (x1pad dies right after) ============
        p2b = ctx.enter_context(tc.tile_pool(name="p2b", bufs=1))
        ctx_mid = ExitStack()
        pmid = ctx_mid.enter_context(tc.tile_pool(name="pmid", bufs=1))
        with tc.tile_pool(name="pin1", bufs=1) as pin1:
            x1pad = pin1.tile([128, 2, 34, 66], F32R, tag="x1pad")
            nc.sync.dma_start(out=x1pad[:, 0], in_=dd["x1b"][:, 0].bitcast(F32R))
            nc.sync.dma_start(out=x1pad[:, 1], in_=dd["x1b"][:, 1].bitcast(F32R))
            x1cp = pmid.tile([128, 4096], F32R, tag="x1cp")
            nc.vector.memset(x1cp[:].bitcast(F32), 0.0)
            acc1 = prow.tile([80, 8], F32, tag="acc1")
            accsq1 = prow.tile([80, 8], F32, tag="accsq1")

            def x1win(c8, dy, dx):
                j, c4 = divmod(c8, 4)
                return x1pad[:, j, c4 * 8 + dy: c4 * 8 + dy + 8, dx:dx + 64]

            conv3x3(wt1, 80, x1win, x1cp, acc1, accsq1)

        a1, c1 = gn_params("1", acc1, accsq1, 80, ws["indic"], ws["indicT"],
                           gn1_g, gn1_b, 1.0 / (16 * HW))
        # GN + SiLU in place -> x1p (rows 0..79 of x1cp)
        nc.scalar.activation(x1cp[0:80, :], x1cp[0:80, :].bitcast(F32), AF.Silu,
                             bias=c1[:], scale=a1[:])

        # ============ stage 1b: x2 side ============
        pin2 = ctx_mid.enter_context(tc.tile_pool(name="pin2", bufs=1))
        x2pad = pin2.tile([128, 2, 34, 66], F32R, tag="x2pad")
        nc.gpsimd.dma_start(out=x2pad[:, 0], in_=dd["x2b"][:, 0].bitcast(F32R))
        nc.gpsimd.dma_start(out=x2pad[:, 1], in_=dd["x2b"][:, 1].bitcast(F32R))

        def x2win(c8):
            j, c4 = divmod(c8, 4)
            return x2pad[:, j, 1 + c4 * 8: 9 + c4 * 8, 1:65]

        # xmin (band order): four 1024-token passes, exact f32 read of x2
        for cq in range(4):
            j, c2 = divmod(cq, 2)
            xw = pscr.tile([80, 16, 64], F32, tag="csq")
            nc.sync.dma_start(out=xw[:], in_=dd["x2b"][0:80, j, 1 + 16 * c2: 17 + 16 * c2, 1:65])
            ng = pscr.tile([80, 1024], F32, tag="csq")
            nc.vector.tensor_scalar(ng[:].rearrange("p (a b) -> p a b", a=16),
                                    xw[:], -1.0, None, ALU.mult)
            ar = pscr.tile([80, 1024], F32, tag="csq")
            nc.gpsimd.partition_all_reduce(ar[:], ng[:], channels=80,
                                           reduce_op=bass_isa.ReduceOp.max)
            xm = pror.tile([1, 4096], F32, tag="rrow", name="xm")
            nc.vector.tensor_scalar(xm[:, 0:1024], ar[0:1, :], -1.0, None, ALU.mult)
            nc.sync.dma_start(out=dd["xmin_o"][None, cq * 1024:(cq + 1) * 1024],
                              in_=xm[:, 0:1024])

        # x2 token LN stats -> xh2
        xh2 = pmid.tile([128, 4096], F32R, tag="xh2")
        nc.vector.memset(xh2[:].bitcast(F32), 0.0)
        r2_row, _ = token_stats("x2", 4096, x2win, False, xh2[80:81, :])
        nc.sync.dma_start(out=xh2[81:82, :], in_=onesrow[None, :].bitcast(F32R))
        for c in range(8):
            sl = slice(c * 512, (c + 1) * 512)
            bc = pscr.tile([80, 512], F32, tag="bscr", name="bc")
            nc.gpsimd.partition_broadcast(bc[:], r2_row[0:1, sl])
            nc.vector.tensor_tensor(xh2[0:80, sl].rearrange("p (a b) -> p a b", a=8),
                                    x2win(c)[0:80].bitcast(F32),
                                    bc[:].rearrange("p (a b) -> p a b", a=8), ALU.mult)

        # k, v projections (LN folded into lhs weights)
        k_sb = p2b.tile([128, 4096], F32R, tag="k_sb")
        nc.vector.memset(k_sb[:].bitcast(F32), 0.0)
        v_sb = pmid.tile([128, 4096], F32, tag="v_sb")
        nc.vector.memset(v_sb[:], 0.0)
        for c in range(8):
            sl = slice(c * 512, (c + 1) * 512)
            ps_k = psm.tile([80, 512], F32, tag="cps")
            nc.tensor.matmul(ps_k[:], ws["lhsk"], xh2[:, sl], start=True, stop=True)
            nc.vector.tensor_copy(k_sb[0:80, sl], ps_k[:])
            ps_v = psm.tile([80, 512], F32, tag="cps")
            nc.tensor.matmul(ps_v[:], ws["lhsv"], xh2[:, sl], start=True, stop=True)
            nc.vector.tensor_copy(v_sb[0:80, sl], ps_v[:])

        # gate branch: conv3x3 -> GN+SiLU (in place) -> 1x1 -> sigmoid -> gate v
        gcp = pmid.tile([128, 4096], F32R, tag="gcp")
        nc.vector.memset(gcp[:].bitcast(F32), 0.0)
        accg = prow.tile([40, 8], F32, tag="accg")
        accsqg = prow.tile([40, 8], F32, tag="accsqg")

        def gwin(c8, dy, dx):
            j, c4 = divmod(c8, 4)
            return x2pad[:, j, c4 * 8 + dy: c4 * 8 + dy + 8, dx:dx + 64]

        conv3x3(wtg, 40, gwin, gcp, accg, accsqg)
        ag, cg = gn_params("g", accg, accsqg, 40, ws["indicg"], ws["indicgT"],
                           gng_g, gng_b, 1.0 / (8 * HW))
        nc.scalar.activation(gcp[0:40, :], gcp[0:40, :].bitcast(F32), AF.Silu,
                             bias=cg[:], scale=ag[:])
        for c in range(8):
            sl = slice(c * 512, (c + 1) * 512)
            ps_g2 = psm.tile([80, 512], F32, tag="cps")
            nc.tensor.matmul(ps_g2[:], ws["wg2"], gcp[:, sl], start=True, stop=True)
            sg = pscr.tile([80, 512], F32, tag="bscr")
            nc.scalar.activation(sg[:], ps_g2[:], AF.Sigmoid, bias=bg2_ap)
            nc.vector.tensor_scalar(sg[:], sg[:], GATE_SCALE, 1.0, ALU.mult, ALU.add)
            nc.vector.tensor_tensor(v_sb[0:80, sl], v_sb[0:80, sl], sg[:], ALU.mult)

        # x2_id: conv1x1 + GN (store first half only; stats from accums)
        idc = p2b.tile([80, HALF], F32, tag="idc")
        accid = prow.tile([80, 8], F32, tag="accid")
        accsqid = prow.tile([80, 8], F32, tag="accsqid")
        for c in range(8):
            ps_id = psm.tile([80, 512], F32, tag="cps")
            nc.tensor.matmul(ps_id[:], ws["wid"], x2win(c), start=True, stop=True)
            if c < 4:
                out_ap = idc[:, c * 512:(c + 1) * 512]
            else:
                idscr = pscr.tile([80, 512], F32, tag="bscr", name="idscr")
                out_ap = idscr[:]
            nc.scalar.activation(out_ap, ps_id[:], AF.Copy, accum_out=accid[:, c:c + 1])
            sqi = pscr.tile([128, 512], F32, tag="csq")
            nc.scalar.activation(sqi[0:80, :], ps_id[:], AF.Square,
                                 accum_out=accsqid[:, c:c + 1])
        aid, cid = gn_params("id", accid, accsqid, 80, ws["indic"], ws["indicT"],
                             gnid_g, gnid_b, 1.0 / (16 * HW))
        nc.vector.tensor_scalar(idc[:], idc[:], aid[:], cid[:], ALU.mult, ALU.add)

        # vT: token-major gated v in cols 0..79, ones in col 96 (denominator row)
        vT = p2b.tile([128, 32, 97], F32R, tag="vT")
        nc.vector.memset(vT[:].bitcast(F32), 1.0)
        for kc in range(32):
            ps_t = psm.tile([128, 128], F32, tag="cps")
            nc.tensor.transpose(ps_t[:], v_sb[:, kc * 128:(kc + 1) * 128], ident_sb)
            nc.vector.tensor_copy(vT[:, kc, 0:80], ps_t[:, 0:80])

        # x1 token LN stats (first HALF only) -> xh1 -> q
        xh1 = pmid.tile([128, HALF], F32R, tag="xh1")
        nc.vector.memset(xh1[:].bitcast(F32), 0.0)
        r1_row, _ = token_stats(
            "x1", HALF,
            lambda c: x1cp[:, c * 512:(c + 1) * 512].rearrange("p (a b) -> p a b", a=8),
            False, xh1[80:81, :])
        nc.sync.dma_start(out=xh1[81:82, :], in_=onesrow[None, 0:HALF].bitcast(F32R))
        for c in range(4):
            sl = slice(c * 512, (c + 1) * 512)
            bc = pscr.tile([80, 512], F32, tag="bscr", name="bc")
            nc.gpsimd.partition_broadcast(bc[:], r1_row[0:1, sl])
            nc.vector.tensor_tensor(xh1[0:80, sl], x1cp[0:80, sl].bitcast(F32),
                                    bc[:], ALU.mult)
        q_sb = p2b.tile([128, HALF], F32R, tag="q_sb")
        nc.vector.memset(q_sb[:].bitcast(F32), 0.0)
        for c in range(4):
            sl = slice(c * 512, (c + 1) * 512)
            ps_q = psm.tile([80, 512], F32, tag="cps")
            nc.tensor.matmul(ps_q[:], ws["lhsq"], xh1[:, sl], start=True, stop=True)
            nc.vector.tensor_copy(q_sb[0:80, sl], ps_q[:])

        ctx_mid.close()

        # ============ stage 2: attention ============
        yres = p2b.tile([128, HALF], F32R, tag="yres")
        nc.vector.memset(yres[:].bitcast(F32), 0.0)
        for qc in range(4):
            qsl = slice(qc * 512, (qc + 1) * 512)
            ps_y = psm.tile([97, 512], F32, tag="cps")
            for k2 in range(16):
                ps_s = psm.tile([128, 1024], F32, tag="scps", name="ps_s")
                nc.tensor.matmul(ps_s[:, 0:512], k_sb[:, (2 * k2) * 128:(2 * k2 + 1) * 128],
                                 q_sb[:, qsl], start=True, stop=True)
                nc.tensor.matmul(ps_s[:, 512:1024], k_sb[:, (2 * k2 + 1) * 128:(2 * k2 + 2) * 128],
                                 q_sb[:, qsl], start=True, stop=True)
                e_sb = pscr.tile([128, 1024], F32R, tag="e_sb")
                nc.scalar.activation(e_sb[:], ps_s[:], AF.Exp, scale=ISQK)
                nc.tensor.matmul(ps_y[:], vT[:, 2 * k2, :], e_sb[:, 0:512],
                                 start=(k2 == 0), stop=False)
                nc.tensor.matmul(ps_y[:], vT[:, 2 * k2 + 1, :], e_sb[:, 512:1024],
                                 start=False, stop=(k2 == 15))
            rec = pscr.tile([1, 512], F32, tag="bscr")
            nc.vector.reciprocal(rec[:], ps_y[96:97, :])
            rb = pscr.tile([80, 512], F32, tag="bscr")
            nc.gpsimd.partition_broadcast(rb[:], rec[:])
            nc.vector.tensor_tensor(yres[0:80, qsl], ps_y[0:80, :], rb[:], ALU.mult)
            nc.vector.tensor_tensor(yres[0:80, qsl], yres[0:80, qsl].bitcast(F32),
                                    idc[:, qsl], ALU.add)

            # out-LN on this 512-token block: stats stay on one partition
            # (no DRAM bounce), then pairwise gather overlapping the next qc
            src = yres[:, qsl].rearrange("p (a b) -> p a b", a=8)
            ps_s = psm.tile([1, 512], F32, tag="cps", name="ps_s_o")
            nc.tensor.matmul(ps_s[:], ones_col[:], src, start=True, stop=True)
            sqo = pscr.tile([128, 8, 64], F32R, tag="stq", name="sqo")
            nc.scalar.activation(sqo[:], src.bitcast(F32), AF.Square)
            ps_q = psm.tile([1, 512], F32, tag="cps", name="ps_q_o")
            nc.tensor.matmul(ps_q[:], ones_col[:], sqo[:], start=True, stop=True)
            t_m = pscr.tile([1, 512], F32, tag="tm")
            nc.vector.tensor_scalar(t_m[:], ps_s[:], 1.0 / K, None, ALU.mult)
            t_r = pscr.tile([1, 512], F32, tag="tr")
            nc.vector.tensor_scalar(t_r[:], ps_q[:], 1.0 / K, None, ALU.mult)
            t_n = pscr.tile([1, 512], F32, tag="tn")
            nc.vector.tensor_tensor(t_n[:], t_m[:], t_m[:], ALU.mult)
            nc.vector.tensor_tensor(t_r[:], t_r[:], t_n[:], ALU.subtract)
            nc.scalar.activation(t_r[:], t_r[:], AF.Sqrt, bias=eps_col[0:1, :])
            nc.vector.reciprocal(t_r[:], t_r[:])
            nc.vector.tensor_tensor(t_n[:], t_r[:], t_m[:], ALU.mult)
            nc.vector.tensor_scalar(t_n[:], t_n[:], -1.0, None, ALU.mult)
            bc = pscr.tile([80, 512], F32, tag="bscr", name="bc")
            nc.gpsimd.partition_broadcast(bc[:], t_r[:])
            mbc = pscr.tile([80, 512], F32, tag="bscr", name="mbc")
            nc.gpsimd.partition_broadcast(mbc[:], t_n[:])
            yl = pscr.tile([80, 512], F32, tag="csq")
            nc.vector.tensor_tensor(yl[:], yres[0:80, qsl].bitcast(F32), bc[:], ALU.mult)
            nc.vector.tensor_tensor(yl[:], yl[:], mbc[:], ALU.add)
            nc.vector.tensor_scalar(yl[:], yl[:], lno_g, lno_b, ALU.mult, ALU.add)
            nc.sync.dma_start(out=dd["cc_in"][qc], in_=yl[:])
            nc.gpsimd.collective_compute(
                "AllGather", ALU.bypass,
                replica_groups=[[0, 1], [2, 3], [4, 5], [6, 7]],
                ins=[dd["cc_in"][qc][:]], outs=[dd["cc_out"][qc][:]],
            )

        ph = ctx.enter_context(tc.tile_pool(name="ph", bufs=1))
        ypad = ph.tile([128, 66, 66], F32R, tag="ypad")
        nc.vector.memset(ypad[:].bitcast(F32), 0.0)
        for qc in range(4):
            for r in range(2):
                nc.sync.dma_start(
                    out=ypad[0:80, 1 + 32 * r + 8 * qc: 9 + 32 * r + 8 * qc, 1:65],
                    in_=dd["cc_out"][qc, r].rearrange("p (a b) -> p a b", a=8).bitcast(F32R))

        hcp = ph.tile([128, 4096], F32R, tag="hcp")
        nc.vector.memset(hcp[:].bitcast(F32), 0.0)
        acch = prow.tile([80, 8], F32, tag="acch")
        accsqh = prow.tile([80, 8], F32, tag="accsqh")

        def hwin(c8, dy, dx):
            return ypad[:, c8 * 8 + dy: c8 * 8 + dy + 8, dx:dx + 64]

        conv3x3(wth, 80, hwin, hcp, acch, accsqh)
        ah, ch_ = gn_params("h", acch, accsqh, 80, ws["indic"], ws["indicT"],
                            gnh_g, gnh_b, 1.0 / (16 * HW))
        nc.scalar.activation(hcp[0:80, :], hcp[0:80, :].bitcast(F32), AF.Silu,
                             bias=ch_[:], scale=ah[:])
        pr = pror.tile([1, 4096], F32, tag="rrow", name="pr")
        for c in range(8):
            sl = slice(c * 512, (c + 1) * 512)
            ps_p = psm.tile([1, 512], F32, tag="pstat")
            nc.tensor.matmul(ps_p[:], ws["wh2"], hcp[:, sl], start=True, stop=True)
            nc.scalar.activation(pr[:, sl], ps_p[:], AF.Sigmoid, bias=bh2_ap)
        nc.sync.dma_start(out=dd["pred_o"][None, :], in_=pr[:])


def _prep_inputs(x1, x2, w_p1, gn1_g, gn1_b, w_id, gnid_g, gnid_b, wq, wk, wv,
                 wg1, gng_g, gng_b, wg2, bg2, lnx1_g, lnx1_b, lnx2_g, lnx2_b,
                 lno_g, lno_b, wh1, gnh_g, gnh_b, wh2, bh2):
    f = np.float32

    def band_pack(x, C):
        out = {}
        for h in (0, 1):
            arr = np.zeros((128, 2, 34, 66), f)
            for j in range(2):
                hj = h if j == 0 else 1 - h
                g0 = 32 * hj - 1
                lo, hi = max(0, g0), min(64, g0 + 34)
                arr[:C, j, lo - g0: hi - g0, 1:65] = x[:, lo:hi, :]
            out[h] = arr
        return out

    def fold_lhs(wmat, lng, lnb):
        wmat = np.asarray(wmat).astype(f)
        wp = wmat * np.asarray(lng).astype(f)[None, :]
        lhs = np.zeros((128, 80), f)
        lhs[0:80, :] = wp.T
        lhs[80, :] = -wp.sum(axis=1)
        lhs[81, :] = wmat @ np.asarray(lnb).astype(f)
        return lhs

    def taps(wc, cin, cout):
        wt = np.zeros((9, 128, cout), f)
        wc = np.asarray(wc).astype(f)
        for t in range(9):
            dy, dx = divmod(t, 3)
            wt[t, 0:cin, :] = wc[:, :, dy, dx].T
        return wt

    wid_t = np.zeros((128, 80), f); wid_t[0:80, :] = np.asarray(w_id)[:, :, 0, 0].T
    wg2_t = np.zeros((128, 80), f); wg2_t[0:40, :] = np.asarray(wg2)[:, :, 0, 0].T
    wh2_t = np.zeros((128, 1), f); wh2_t[0:80, 0] = np.asarray(wh2)[0, :, 0, 0]
    indic = np.zeros((128, 5), f); indicT = np.zeros((128, 80), f)
    for ch in range(80):
        indic[ch, ch // 16] = 1.0
        indicT[ch // 16, ch] = 1.0
    indicg = np.zeros((128, 5), f); indicgT = np.zeros((128, 40), f)
    for ch in range(40):
        indicg[ch, ch // 8] = 1.0
        indicgT[ch // 8, ch] = 1.0
    vecs = np.zeros((128, 12), f)
    for col, v, n in ((0, gn1_g, 80), (1, gn1_b, 80), (2, gnid_g, 80), (3, gnid_b, 80),
                     (4, gng_g, 40), (5, gng_b, 40), (6, gnh_g, 80), (7, gnh_b, 80),
                     (8, lno_g, 80), (9, lno_b, 80), (10, bg2, 80), (11, bh2, 1)):
        vecs[0:n, col] = np.asarray(v).astype(f)

    wpack = np.concatenate([
        taps(w_p1, C1, 80).transpose(1, 0, 2).reshape(128, 720),
        taps(wg1, 80, 40).transpose(1, 0, 2).reshape(128, 360),
        taps(wh1, 80, 80).transpose(1, 0, 2).reshape(128, 720),
        wid_t, wg2_t, wh2_t,
        fold_lhs(wq, lnx1_g, lnx1_b), fold_lhs(wk, lnx2_g, lnx2_b),
        fold_lhs(wv, lnx2_g, lnx2_b),
        indic, indicT, indicg, indicgT, vecs,
        np.eye(128, dtype=f)], axis=1).astype(f)
    shared = dict(wpack=wpack, onesrow=np.ones(4096, f))
    in_maps = []
    for core in range(8):
        b, h = divmod(core, 2)
        in_maps.append(dict(
            x1b=band_pack(np.asarray(x1)[b].astype(f), C1)[h],
            x2b=band_pack(np.asarray(x2)[b].astype(f), K)[h],
            **shared))
    return in_maps


def kernel(**inputs):
    global _BUILT
    if _BUILT is None:
        _BUILT = _build()
    in_maps = _prep_inputs(**inputs)
    last_err = None
    for _ in range(3):
        try:
            res = run_bass_kernel_spmd(_BUILT, in_maps, list(range(8))).results
            break
        except Exception as e:  # transient axon worker hangups
            last_err = e
    else:
        raise last_err
    pred = np.stack([res[2 * b]["pred"] for b in range(B)]).astype(np.float32)
    xmin = np.stack([res[2 * b]["xminv"] for b in range(B)]).astype(np.float32)
    return pred, xmin



# revision 12
# speedup vs baseline: 1.0684x; 1.0684x over previous
"""Trainium2 Bass kernel for nn_Expert_layer2 (dense per-sample HWxHW attention block).

Sharding: 8 cores = 4 samples x 2 query-halves. Each core receives its sample's
inputs in *band order* (band 0 = the core's query-half rows, band 1 = the other
half), computes the conv/GN/LN pre-projections and the attention for its 2048
queries over all 4096 keys, pair-AllGathers the normalized attention output,
then runs the conv head redundantly in global row order. Host takes pred/xmin
from the even core of each pair.

v2 notes:
- x1 conv packs 2 taps per matmul (host duplicates x1 column-shifted into
  partitions 64:128): 48 matmuls instead of 72.
- gate conv3x3 and x2_id conv1x1 merged into one 120-wide matmul group.
- no full-tile memsets: matmuls contract only the live partition rows.
- act tables: Sqrt region -> Silu/Tanh region -> Ln/Exp region (attention,
  rsqrt via exp(-0.5*ln(v+eps))) -> Silu/Tanh region (head). 4 loads total.
- gate sigmoid folded into tanh (+ lhsv pre-scaled by GATE_SCALE/2), pred
  sigmoid via tanh.
"""

from contextlib import ExitStack

import numpy as np
import concourse.bass as bass
from concourse import bacc
import concourse.tile as tile
import concourse.mybir as mybir
import concourse.bass_isa as bass_isa
from concourse.bass_utils import run_bass_kernel_spmd

F32 = mybir.dt.float32
F32R = mybir.dt.float32r
AF = mybir.ActivationFunctionType
ALU = mybir.AluOpType

B, C1, K, H, W = 4, 64, 80, 64, 64
HW = H * W
HALF = HW // 2
EPS = 1e-5
GATE_SCALE = 0.1
ISQK = float(1.0 / np.sqrt(np.float32(K)))

_BUILT = None


def _build():
    nc = bacc.Bacc("TRN2", target_bir_lowering=False, num_devices=8)

    dd = {}
    dd["x1b"] = nc.dram_tensor("x1b", [128, 2, 34, 66], F32, kind="ExternalInput")
    dd["x2b"] = nc.dram_tensor("x2b", [128, 2, 34, 66], F32, kind="ExternalInput")
    dd["wpack"] = nc.dram_tensor("wpack", [128, 2231], F32, kind="ExternalInput")
    dd["onesrow"] = nc.dram_tensor("onesrow", [4096], F32, kind="ExternalInput")
    dd["pred_o"] = nc.dram_tensor("pred", [4096], F32, kind="ExternalOutput")
    dd["xmin_o"] = nc.dram_tensor("xminv", [4096], F32, kind="ExternalOutput")
    dd["cc_in"] = nc.dram_tensor("cc_in", [4, 80, 512], F32)
    dd["cc_out"] = nc.dram_tensor("cc_out", [4, 2, 80, 512], F32)

    with tile.TileContext(nc) as tc:
        _body(nc, tc, dd)
    nc.finalize()
    return nc


def _body(nc, tc, dd):
    onesrow = dd["onesrow"]
    ctx = ExitStack()
    with ctx:
        pw = ctx.enter_context(tc.tile_pool(name="pw", bufs=1))
        prow = ctx.enter_context(tc.tile_pool(name="prow", bufs=1))
        pscr = ctx.enter_context(tc.tile_pool(name="pscr", bufs=2))
        pror = ctx.enter_context(tc.tile_pool(name="pror", bufs=1))
        psm = ctx.enter_context(tc.tile_pool(name="psm", bufs=2, space="PSUM"))

        # ------------ persistent weights (one packed DMA) ------------
        wpk = pw.tile([128, 2231], F32R, tag="wpk")
        nc.sync.dma_start(out=wpk[:, 0:744], in_=dd["wpack"][:, 0:744].bitcast(F32R))
        nc.gpsimd.dma_start(out=wpk[:, 744:1488], in_=dd["wpack"][:, 744:1488].bitcast(F32R))
        nc.sync.dma_start(out=wpk[:, 1488:2231], in_=dd["wpack"][:, 1488:2231].bitcast(F32R))
        off = [0]

        def _wslice(n):
            a = wpk[:, off[0]:off[0] + n]
            off[0] += n
            return a

        wt1 = _wslice(480).rearrange("p (t f) -> p t f", t=6)
        wtg = _wslice(360).rearrange("p (t f) -> p t f", t=9)
        wth = _wslice(720).rearrange("p (t f) -> p t f", t=9)
        ws = {}
        for nm, fd in (("wid", 80), ("wg2", 80), ("wh2", 1), ("lhsq", 80), ("lhsk", 80),
                       ("lhsv", 80), ("indic", 5), ("indicT", 80), ("indicg", 5),
                       ("indicgT", 40)):
            ws[nm] = _wslice(fd)
        vec_sb = _wslice(12).bitcast(F32)
        ident_sb = _wslice(128)
        assert off[0] == 2231
        ones_col = pw.tile([128, 1], F32R, tag="onescol")
        nc.sync.dma_start(out=ones_col[:], in_=onesrow[0:128, None].bitcast(F32R))
        eps_col = pw.tile([128, 1], F32, tag="epscol")
        nc.vector.memset(eps_col[:], EPS)

        gn1_g, gn1_b = vec_sb[0:80, 0:1], vec_sb[0:80, 1:2]
        gnid_g, gnid_b = vec_sb[0:80, 2:3], vec_sb[0:80, 3:4]
        gng_g, gng_b = vec_sb[0:40, 4:5], vec_sb[0:40, 5:6]
        gnh_g, gnh_b = vec_sb[0:80, 6:7], vec_sb[0:80, 7:8]
        lno_g, lno_b = vec_sb[0:80, 8:9], vec_sb[0:80, 9:10]
        bg2h_ap = vec_sb[0:80, 10:11]
        bh2h_ap = vec_sb[0:1, 11:12]

        # ============ helpers ============
        def token_stats(nm, ntok, chunk_fn, mr_target_ap, lnexp):
            """chunk_fn(c) -> f32r AP [80, 8, 64] of 512 tokens.
            Writes r*m row to mr_target_ap (f32r), returns r_row [1, ntok]."""
            nchunk = ntok // 512
            dn = nc.dram_tensor(f"dn_{nm}", [2, ntok], F32)
            up = nc.dram_tensor(f"up_{nm}", [2, ntok], F32)
            for c in range(nchunk):
                src = chunk_fn(c)
                sqs = pscr.tile([1, 1024], F32, tag="csq")
                ps_s = psm.tile([1, 512], F32, tag="pstat")
                nc.tensor.matmul(ps_s[:], ones_col[0:80], src, start=True, stop=True)
                nc.vector.tensor_copy(sqs[:, 0:512], ps_s[:])
                sq = pscr.tile([80, 8, 64], F32R, tag="stq")
                nc.scalar.activation(sq[:], src.bitcast(F32), AF.Square)
                ps_q = psm.tile([1, 512], F32, tag="pstat")
                nc.tensor.matmul(ps_q[:], ones_col[0:80], sq[:], start=True, stop=True)
                nc.scalar.copy(sqs[:, 512:1024], ps_q[:])
                nc.sync.dma_start(out=dn[:, c * 512:(c + 1) * 512], in_=sqs[:])
            ncol = ntok // 128
            st = prow.tile([128, 2, ncol], F32, tag=f"stt_{nm}")
            nc.sync.dma_start(out=st[:], in_=dn.rearrange("r (p c) -> p r c", p=128))
            m = prow.tile([128, ncol], F32, tag=f"m_{nm}")
            nc.vector.tensor_scalar(m[:], st[:, 0, :], 1.0 / K, None, ALU.mult)
            msq = prow.tile([128, ncol], F32, tag=f"ms_{nm}")
            nc.vector.tensor_tensor(msq[:], m[:], m[:], ALU.mult)
            var = prow.tile([128, ncol], F32, tag=f"v_{nm}")
            nc.vector.tensor_scalar(var[:], st[:, 1, :], 1.0 / K, None, ALU.mult)
            nc.vector.tensor_tensor(var[:], var[:], msq[:], ALU.subtract)
            rup = prow.tile([128, 2, ncol], F32, tag=f"ru_{nm}")
            if lnexp:
                nc.scalar.activation(var[:], var[:], AF.Ln, bias=eps_col[:])
                nc.scalar.activation(rup[:, 0, :], var[:], AF.Exp, scale=-0.5)
            else:
                nc.scalar.activation(var[:], var[:], AF.Sqrt, bias=eps_col[:])
                nc.vector.reciprocal(rup[:, 0, :], var[:])
            nc.vector.tensor_tensor(rup[:, 1, :], rup[:, 0, :], m[:], ALU.mult)
            nc.sync.dma_start(out=up.rearrange("r (p c) -> p r c", p=128), in_=rup[:])
            r_row = pror.tile([1, 4096], F32, tag="rrow", name="r_row")[:, 0:ntok]
            nc.sync.dma_start(out=r_row, in_=up[0, None, :])
            nc.sync.dma_start(out=mr_target_ap, in_=up[1, None, :].bitcast(F32R))
            return r_row

        def evac(nout, ps, out_ap, acc, accsq, c8):
            """PSUM->SBUF copy + square, both with per-chunk channel accums."""
            nc.scalar.activation(out_ap, ps[:], AF.Copy, accum_out=acc[:, c8:c8 + 1])
            sq = pscr.tile([120, 512], F32, tag="csq")
            nc.scalar.activation(sq[0:nout, :], ps[:], AF.Square,
                                 accum_out=accsq[:, c8:c8 + 1])

        def gn_params(nm, acc, accsq, nch, ind_sb, indT_sb, gamma, beta, inv_n, lnexp):
            s_ch = prow.tile([nch, 2], F32R, tag=f"sch_{nm}")
            with nc.allow_low_precision(reason="f32r == f32 bits; feeds f32r matmul"):
                nc.vector.tensor_reduce(s_ch[:, 0:1], acc[:], mybir.AxisListType.X, ALU.add)
                nc.vector.tensor_reduce(s_ch[:, 1:2], accsq[:], mybir.AxisListType.X, ALU.add)
            ps_g = psm.tile([5, 2], F32, tag="pstat")
            nc.tensor.matmul(ps_g[:], ind_sb[0:nch, :], s_ch[:], start=True, stop=True)
            gst = prow.tile([5, 2], F32R, tag=f"gst_{nm}")
            nc.vector.tensor_scalar(gst[:, 0:1], ps_g[:, 0:1], inv_n, None, ALU.mult)
            msq = prow.tile([5, 1], F32, tag=f"msq_{nm}")
            nc.vector.tensor_tensor(msq[:], gst[:, 0:1], gst[:, 0:1], ALU.mult)
            vr = prow.tile([5, 1], F32, tag=f"vr_{nm}")
            nc.vector.tensor_scalar(vr[:], ps_g[:, 1:2], inv_n, None, ALU.mult)
            nc.vector.tensor_tensor(vr[:], vr[:], msq[:], ALU.subtract)
            if lnexp:
                nc.scalar.activation(vr[:], vr[:], AF.Ln, bias=eps_col[0:5, :])
                nc.scalar.activation(gst[:, 1:2], vr[:], AF.Exp, scale=-0.5)
            else:
                nc.scalar.activation(vr[:], vr[:], AF.Sqrt, bias=eps_col[0:5, :])
                with nc.allow_low_precision(reason="f32r == f32 bits"):
                    nc.vector.reciprocal(gst[:, 1:2], vr[:])
            ps_bc = psm.tile([nch, 2], F32, tag="pstat")
            nc.tensor.matmul(ps_bc[:], indT_sb[0:5, 0:nch], gst[:], start=True, stop=True)
            a = prow.tile([nch, 1], F32, tag=f"a_{nm}")
            c = prow.tile([nch, 1], F32, tag=f"c_{nm}")
            nc.vector.tensor_tensor(a[:], gamma, ps_bc[:, 1:2], ALU.mult)
            nc.vector.tensor_tensor(c[:], ps_bc[:, 0:1], a[:], ALU.mult)
            nc.vector.tensor_tensor(c[:], beta, c[:], ALU.subtract)
            return a, c

        # ============ stage 1a: x1 conv (paired taps) ============
        p2b = ctx.enter_context(tc.tile_pool(name="p2b", bufs=1))
        ctx_mid = ExitStack()
        pmid = ctx_mid.enter_context(tc.tile_pool(name="pmid", bufs=1))
        x1cp = pmid.tile([80, 4096], F32R, tag="x1cp")
        acc1 = prow.tile([80, 8], F32, tag="acc1")
        accsq1 = prow.tile([80, 8], F32, tag="accsq1")
        with tc.tile_pool(name="pin1", bufs=1) as pin1:
            x1pad = pin1.tile([128, 2, 34, 66], F32R, tag="x1pad")
            nc.sync.dma_start(out=x1pad[:, 0], in_=dd["x1b"][:, 0].bitcast(F32R))
            nc.sync.dma_start(out=x1pad[:, 1], in_=dd["x1b"][:, 1].bitcast(F32R))
            for c8 in range(8):
                j, c4 = divmod(c8, 4)
                ps = psm.tile([80, 512], F32, tag="cps")
                for dy in range(3):
                    r0 = c4 * 8 + dy
                    # pair: taps (dy,0)+(dy,1) via column-shifted copy in rows 64:128
                    nc.tensor.matmul(ps[:], wt1[:, 2 * dy, :],
                                     x1pad[:, j, r0:r0 + 8, 0:64],
                                     start=(dy == 0), stop=False)
                    # single: tap (dy,2), rows 0:64 only
                    nc.tensor.matmul(ps[:], wt1[0:64, 2 * dy + 1, :],
                                     x1pad[0:64, j, r0:r0 + 8, 2:66],
                                     start=False, stop=(dy == 2))
                evac(80, ps, x1cp[:, c8 * 512:(c8 + 1) * 512], acc1, accsq1, c8)

        a1, c1 = gn_params("1", acc1, accsq1, 80, ws["indic"], ws["indicT"],
                           gn1_g, gn1_b, 1.0 / (16 * HW), lnexp=False)

        # ============ stage 1b: x2 side ============
        pin2 = ctx_mid.enter_context(tc.tile_pool(name="pin2", bufs=1))
        x2pad = pin2.tile([128, 2, 34, 66], F32R, tag="x2pad")
        nc.gpsimd.dma_start(out=x2pad[:, 0], in_=dd["x2b"][:, 0].bitcast(F32R))
        nc.gpsimd.dma_start(out=x2pad[:, 1], in_=dd["x2b"][:, 1].bitcast(F32R))

        def x2win(c8):
            j, c4 = divmod(c8, 4)
            return x2pad[0:80, j, 1 + c4 * 8: 9 + c4 * 8, 1:65]

        # xmin (band order): four 1024-token passes, exact f32 read of x2
        for cq in range(4):
            j, c2 = divmod(cq, 2)
            xw = pscr.tile([80, 16, 64], F32, tag="csq")
            nc.sync.dma_start(out=xw[:], in_=dd["x2b"][0:80, j, 1 + 16 * c2: 17 + 16 * c2, 1:65])
            ng = pscr.tile([80, 1024], F32, tag="csq")
            nc.vector.tensor_scalar(ng[:].rearrange("p (a b) -> p a b", a=16),
                                    xw[:], -1.0, None, ALU.mult)
            ar = pscr.tile([80, 1024], F32, tag="csq")
            nc.gpsimd.partition_all_reduce(ar[:], ng[:], channels=80,
                                           reduce_op=bass_isa.ReduceOp.max)
            xm = pror.tile([1, 4096], F32, tag="rrow", name="xm")
            nc.vector.tensor_scalar(xm[:, 0:1024], ar[0:1, :], -1.0, None, ALU.mult)
            nc.sync.dma_start(out=dd["xmin_o"][None, cq * 1024:(cq + 1) * 1024],
                              in_=xm[:, 0:1024])

        # x2 token LN stats -> xh2 [82, 4096]
        xh2 = pmid.tile([82, 4096], F32R, tag="xh2")
        r2_row = token_stats("x2", 4096, x2win, xh2[80:81, :], lnexp=False)
        nc.sync.dma_start(out=xh2[81:82, :], in_=onesrow[None, :].bitcast(F32R))

        # gate conv3x3
        gcp = pmid.tile([40, 4096], F32R, tag="gcp")
        accg = prow.tile([40, 8], F32, tag="accg")
        accsqg = prow.tile([40, 8], F32, tag="accsqg")
        for c8 in range(8):
            j, c4 = divmod(c8, 4)
            ps = psm.tile([40, 512], F32, tag="cps")
            for tap in range(9):
                dy, dx = divmod(tap, 3)
                nc.tensor.matmul(ps[:], wtg[0:80, tap, :],
                                 x2pad[0:80, j, c4 * 8 + dy: c4 * 8 + dy + 8, dx:dx + 64],
                                 start=(tap == 0), stop=(tap == 8))
            evac(40, ps, gcp[:, c8 * 512:(c8 + 1) * 512], accg, accsqg, c8)
        ag, cg = gn_params("g", accg, accsqg, 40, ws["indicg"],
                           ws["indicgT"], gng_g, gng_b, 1.0 / (8 * HW), lnexp=False)

        # x2_id conv1x1 (store first half only; stats from accums)
        idc = p2b.tile([80, HALF], F32, tag="idc")
        accid = prow.tile([80, 8], F32, tag="accid")
        accsqid = prow.tile([80, 8], F32, tag="accsqid")
        for c in range(8):
            ps_id = psm.tile([80, 512], F32, tag="cps")
            nc.tensor.matmul(ps_id[:], ws["wid"][0:80, :], x2win(c), start=True, stop=True)
            if c < 4:
                out_ap = idc[:, c * 512:(c + 1) * 512]
            else:
                idscr = pscr.tile([80, 512], F32, tag="bscr", name="idscr")
                out_ap = idscr[:]
            evac(80, ps_id, out_ap, accid, accsqid, c)
        aid, cid = gn_params("id", accid, accsqid, 80, ws["indic"],
                             ws["indicT"], gnid_g, gnid_b, 1.0 / (16 * HW), lnexp=False)

        # ---- silu/tanh act-table region starts here ----
        # GN + SiLU on x1p (own half only) and gate rows
        nc.scalar.activation(x1cp[0:80, 0:HALF], x1cp[0:80, 0:HALF].bitcast(F32),
                             AF.Silu, bias=c1[:], scale=a1[:])
        nc.scalar.activation(gcp[:, :], gcp[:, :].bitcast(F32), AF.Silu,
                             bias=cg[:], scale=ag[:])
        # idc: GN apply on own half only (DVE)
        nc.vector.tensor_scalar(idc[:], idc[:], aid[:], cid[:], ALU.mult, ALU.add)

        # xh2 rows 0:80 = x2 * r (token LN, gamma/beta folded into lhs weights)
        for c in range(8):
            sl = slice(c * 512, (c + 1) * 512)
            bc = pscr.tile([80, 512], F32, tag="bscr", name="bc")
            nc.gpsimd.partition_broadcast(bc[:], r2_row[0:1, sl])
            nc.vector.tensor_tensor(xh2[0:80, sl].rearrange("p (a b) -> p a b", a=8),
                                    x2win(c).bitcast(F32),
                                    bc[:].rearrange("p (a b) -> p a b", a=8), ALU.mult)

        # k, v projections (LN folded into lhs weights; lhsv pre-scaled 0.05)
        k_sb = p2b.tile([80, 4096], F32R, tag="k_sb")
        v_sb = pmid.tile([80, 4096], F32R, tag="v_sb")
        for c in range(8):
            sl = slice(c * 512, (c + 1) * 512)
            ps_k = psm.tile([80, 512], F32, tag="cps")
            nc.tensor.matmul(ps_k[:], ws["lhsk"][0:82, :], xh2[:, sl], start=True, stop=True)
            nc.vector.tensor_copy(k_sb[:, sl], ps_k[:])
            ps_v = psm.tile([80, 512], F32, tag="cps")
            nc.tensor.matmul(ps_v[:], ws["lhsv"][0:82, :], xh2[:, sl], start=True, stop=True)
            nc.vector.tensor_copy(v_sb[:, sl], ps_v[:])

        # gate 1x1: tanh trick: v *= (tanh((wg2@g+bg2)/2) + 21) [lhsv scaled 0.05]
        for c in range(8):
            sl = slice(c * 512, (c + 1) * 512)
            ps_g2 = psm.tile([80, 512], F32, tag="cps")
            nc.tensor.matmul(ps_g2[:], ws["wg2"][0:40, :], gcp[:, sl],
                             start=True, stop=True)
            tg = pscr.tile([80, 512], F32, tag="bscr")
            nc.scalar.activation(tg[:], ps_g2[:], AF.Tanh, bias=bg2h_ap, scale=0.5)
            nc.vector.tensor_scalar(tg[:], tg[:], 21.0, None, ALU.add)
            nc.vector.tensor_tensor(v_sb[:, sl], v_sb[:, sl].bitcast(F32), tg[:], ALU.mult)

        # x1 token LN stats (first HALF only) -> xh1 -> q  [squares in silu region]
        xh1 = pmid.tile([82, HALF], F32R, tag="xh1")
        r1_row = token_stats(
            "x1", HALF,
            lambda c: x1cp[:, c * 512:(c + 1) * 512].rearrange("p (a b) -> p a b", a=8),
            xh1[80:81, :], lnexp=True)
        nc.sync.dma_start(out=xh1[81:82, :], in_=onesrow[None, 0:HALF].bitcast(F32R))

        # vT: token-major gated v in cols 0..79, ones in col 96 (denominator row)
        vT = p2b.tile([128, 32, 97], F32R, tag="vT")
        nc.vector.memset(vT[:, :, 96:97].bitcast(F32), 1.0)
        for kc in range(32):
            ps_t = psm.tile([128, 80], F32R, tag="cps")
            nc.tensor.transpose(ps_t[:], v_sb[:, kc * 128:(kc + 1) * 128],
                                ident_sb[0:80, 0:80])
            nc.vector.tensor_copy(vT[:, kc, 0:80], ps_t[:])

        for c in range(4):
            sl = slice(c * 512, (c + 1) * 512)
            bc = pscr.tile([80, 512], F32, tag="bscr", name="bc")
            nc.gpsimd.partition_broadcast(bc[:], r1_row[0:1, sl])
            nc.vector.tensor_tensor(xh1[0:80, sl], x1cp[0:80, sl].bitcast(F32),
                                    bc[:], ALU.mult)
        q_sb = p2b.tile([80, HALF], F32R, tag="q_sb")
        for c in range(4):
            sl = slice(c * 512, (c + 1) * 512)
            ps_q = psm.tile([80, 512], F32, tag="cps")
            nc.tensor.matmul(ps_q[:], ws["lhsq"][0:82, :], xh1[:, sl], start=True, stop=True)
            nc.vector.tensor_copy(q_sb[:, sl], ps_q[:])

        ctx_mid.close()

        # ============ stage 2: attention ============
        yres = p2b.tile([80, HALF], F32R, tag="yres")
        for qc in range(4):
            qsl = slice(qc * 512, (qc + 1) * 512)
            ps_y = psm.tile([97, 512], F32, tag="cps")
            for k2 in range(16):
                ps_s = psm.tile([128, 1024], F32, tag="scps", name="ps_s")
                nc.tensor.matmul(ps_s[:, 0:512], k_sb[:, (2 * k2) * 128:(2 * k2 + 1) * 128],
                                 q_sb[:, qsl], start=True, stop=True)
                nc.tensor.matmul(ps_s[:, 512:1024], k_sb[:, (2 * k2 + 1) * 128:(2 * k2 + 2) * 128],
                                 q_sb[:, qsl], start=True, stop=True)
                e_sb = pscr.tile([128, 1024], F32R, tag="e_sb")
                nc.scalar.activation(e_sb[:], ps_s[:], AF.Exp, scale=ISQK)
                nc.tensor.matmul(ps_y[:], vT[:, 2 * k2, :], e_sb[:, 0:512],
                                 start=(k2 == 0), stop=False)
                nc.tensor.matmul(ps_y[:], vT[:, 2 * k2 + 1, :], e_sb[:, 512:1024],
                                 start=False, stop=(k2 == 15))
            rec = pscr.tile([1, 512], F32, tag="bscr")
            nc.vector.reciprocal(rec[:], ps_y[96:97, :])
            rb = pscr.tile([80, 512], F32, tag="bscr")
            nc.gpsimd.partition_broadcast(rb[:], rec[:])
            nc.vector.tensor_tensor(yres[:, qsl], ps_y[0:80, :], rb[:], ALU.mult)
            nc.vector.tensor_tensor(yres[:, qsl], yres[:, qsl].bitcast(F32),
                                    idc[:, qsl], ALU.add)

            # out-LN on this 512-token block: stats stay on one partition,
            # rsqrt via exp(-0.5*ln(v+eps)) to stay in the exp act table
            src = yres[:, qsl].rearrange("p (a b) -> p a b", a=8)
            ps_s = psm.tile([1, 512], F32, tag="pstat", name="ps_s_o")
            nc.tensor.matmul(ps_s[:], ones_col[0:80], src, start=True, stop=True)
            sqo = pscr.tile([80, 8, 64], F32R, tag="stq", name="sqo")
            nc.scalar.activation(sqo[:], src.bitcast(F32), AF.Square)
            ps_q = psm.tile([1, 512], F32, tag="pstat", name="ps_q_o")
            nc.tensor.matmul(ps_q[:], ones_col[0:80], sqo[:], start=True, stop=True)
            t_m = pscr.tile([1, 512], F32, tag="tm")
            nc.vector.tensor_scalar(t_m[:], ps_s[:], 1.0 / K, None, ALU.mult)
            t_n = pscr.tile([1, 512], F32, tag="tn")
            nc.vector.tensor_tensor(t_n[:], t_m[:], t_m[:], ALU.mult)
            t_r = pscr.tile([1, 512], F32, tag="tr")
            nc.vector.tensor_scalar(t_r[:], ps_q[:], 1.0 / K, None, ALU.mult)
            nc.vector.tensor_tensor(t_r[:], t_r[:], t_n[:], ALU.subtract)
            nc.scalar.activation(t_r[:], t_r[:], AF.Ln, bias=eps_col[0:1, :])
            nc.scalar.activation(t_r[:], t_r[:], AF.Exp, scale=-0.5)
            nc.vector.tensor_tensor(t_n[:], t_r[:], t_m[:], ALU.mult)
            nc.vector.tensor_scalar(t_n[:], t_n[:], -1.0, None, ALU.mult)
            bc = pscr.tile([80, 512], F32, tag="bscr", name="bc")
            nc.gpsimd.partition_broadcast(bc[:], t_r[:])
            mbc = pscr.tile([80, 512], F32, tag="bscr", name="mbc")
            nc.gpsimd.partition_broadcast(mbc[:], t_n[:])
            yl = pscr.tile([80, 512], F32, tag="csq")
            nc.vector.tensor_tensor(yl[:], yres[:, qsl].bitcast(F32), bc[:], ALU.mult)
            nc.vector.tensor_tensor(yl[:], yl[:], mbc[:], ALU.add)
            nc.vector.tensor_scalar(yl[:], yl[:], lno_g, lno_b, ALU.mult, ALU.add)
            nc.sync.dma_start(out=dd["cc_in"][qc], in_=yl[:])
            nc.gpsimd.collective_compute(
                "AllGather", ALU.bypass,
                replica_groups=[[0, 1], [2, 3], [4, 5], [6, 7]],
                ins=[dd["cc_in"][qc][:]], outs=[dd["cc_out"][qc][:]],
            )

        ph = ctx.enter_context(tc.tile_pool(name="ph", bufs=1))
        ypad = ph.tile([80, 66, 66], F32R, tag="ypad")
        nc.vector.memset(ypad[:, :, 0:1].bitcast(F32), 0.0)
        nc.vector.memset(ypad[:, :, 65:66].bitcast(F32), 0.0)
        nc.vector.memset(ypad[:, 0:1, 1:65].bitcast(F32), 0.0)
        nc.vector.memset(ypad[:, 65:66, 1:65].bitcast(F32), 0.0)
        for qc in range(4):
            for r in range(2):
                nc.sync.dma_start(
                    out=ypad[:, 1 + 32 * r + 8 * qc: 9 + 32 * r + 8 * qc, 1:65],
                    in_=dd["cc_out"][qc, r].rearrange("p (a b) -> p a b", a=8).bitcast(F32R))

        hcp = ph.tile([80, 4096], F32R, tag="hcp")
        acch = prow.tile([80, 8], F32, tag="acch")
        accsqh = prow.tile([80, 8], F32, tag="accsqh")
        for c8 in range(8):
            ps = psm.tile([80, 512], F32, tag="cps")
            for tap in range(9):
                dy, dx = divmod(tap, 3)
                nc.tensor.matmul(ps[:], wth[0:80, tap, :],
                                 ypad[:, c8 * 8 + dy: c8 * 8 + dy + 8, dx:dx + 64],
                                 start=(tap == 0), stop=(tap == 8))
            evac(80, ps, hcp[:, c8 * 512:(c8 + 1) * 512], acch, accsqh, c8)
        ah, ch_ = gn_params("h", acch, accsqh, 80, ws["indic"], ws["indicT"],
                            gnh_g, gnh_b, 1.0 / (16 * HW), lnexp=True)
        # ---- final silu/tanh region ----
        nc.scalar.activation(hcp[0:80, :], hcp[0:80, :].bitcast(F32), AF.Silu,
                             bias=ch_[:], scale=ah[:])
        pr = pror.tile([1, 4096], F32, tag="rrow", name="pr")
        for c in range(8):
            sl = slice(c * 512, (c + 1) * 512)
            ps_p = psm.tile([1, 512], F32, tag="pstat")
            nc.tensor.matmul(ps_p[:], ws["wh2"][0:80, :], hcp[:, sl], start=True, stop=True)
            nc.scalar.activation(pr[:, sl], ps_p[:], AF.Tanh, bias=bh2h_ap, scale=0.5)
            nc.vector.tensor_scalar(pr[:, sl], pr[:, sl], 0.5, 0.5, ALU.mult, ALU.add)
        nc.sync.dma_start(out=dd["pred_o"][None, :], in_=pr[:])


def _prep_inputs(x1, x2, w_p1, gn1_g, gn1_b, w_id, gnid_g, gnid_b, wq, wk, wv,
                 wg1, gng_g, gng_b, wg2, bg2, lnx1_g, lnx1_b, lnx2_g, lnx2_b,
                 lno_g, lno_b, wh1, gnh_g, gnh_b, wh2, bh2):
    f = np.float32

    def band_pack(x, C, shift_dup=False):
        out = {}
        for h in (0, 1):
            arr = np.zeros((128, 2, 34, 66), f)
            for j in range(2):
                hj = h if j == 0 else 1 - h
                g0 = 32 * hj - 1
                lo, hi = max(0, g0), min(64, g0 + 34)
                arr[:C, j, lo - g0: hi - g0, 1:65] = x[:, lo:hi, :]
            if shift_dup:
                # rows 64:128 = columns shifted left by one (tap dx+1 source)
                arr[64:128, :, :, 0:65] = arr[0:64, :, :, 1:66]
            out[h] = arr
        return out

    def fold_lhs(wmat, lng, lnb, scale=1.0):
        wmat = np.asarray(wmat).astype(f) * f(scale)
        wp = wmat * np.asarray(lng).astype(f)[None, :]
        lhs = np.zeros((128, 80), f)
        lhs[0:80, :] = wp.T
        lhs[80, :] = -wp.sum(axis=1)
        lhs[81, :] = wmat @ np.asarray(lnb).astype(f)
        return lhs

    # x1 conv taps, paired: slot (dy,0) = taps (dy,0)+(dy,1) stacked 64+64;
    # slot (dy,1) = tap (dy,2) in rows 0:64
    wt1 = np.zeros((128, 6, 80), f)
    wp1 = np.asarray(w_p1).astype(f)
    for dy in range(3):
        wt1[0:64, 2 * dy, :] = wp1[:, :, dy, 0].T
        wt1[64:128, 2 * dy, :] = wp1[:, :, dy, 1].T
        wt1[0:64, 2 * dy + 1, :] = wp1[:, :, dy, 2].T

    wtg = np.zeros((128, 9, 40), f)
    wg1a = np.asarray(wg1).astype(f)
    for t in range(9):
        dy, dx = divmod(t, 3)
        wtg[0:80, t, :] = wg1a[:, :, dy, dx].T
    wid_t = np.zeros((128, 80), f); wid_t[0:80, :] = np.asarray(w_id)[:, :, 0, 0].T

    wth = np.zeros((128, 9, 80), f)
    wh1a = np.asarray(wh1).astype(f)
    for t in range(9):
        dy, dx = divmod(t, 3)
        wth[0:80, t, :] = wh1a[:, :, dy, dx].T

    wg2_t = np.zeros((128, 80), f); wg2_t[0:40, :] = np.asarray(wg2)[:, :, 0, 0].T
    wh2_t = np.zeros((128, 1), f); wh2_t[0:80, 0] = np.asarray(wh2)[0, :, 0, 0]
    indic = np.zeros((128, 5), f); indicT = np.zeros((128, 80), f)
    for ch in range(80):
        indic[ch, ch // 16] = 1.0
        indicT[ch // 16, ch] = 1.0
    indicg = np.zeros((128, 5), f); indicgT = np.zeros((128, 40), f)
    for ch in range(40):
        indicg[ch, ch // 8] = 1.0
        indicgT[ch // 8, ch] = 1.0
    vecs = np.zeros((128, 12), f)
    for col, v, n in ((0, gn1_g, 80), (1, gn1_b, 80), (2, gnid_g, 80), (3, gnid_b, 80),
                     (4, gng_g, 40), (5, gng_b, 40), (6, gnh_g, 80), (7, gnh_b, 80),
                     (8, lno_g, 80), (9, lno_b, 80),
                     (10, 0.5 * np.asarray(bg2), 80), (11, 0.5 * np.asarray(bh2), 1)):
        vecs[0:n, col] = np.asarray(v).astype(f)

    wpack = np.concatenate([
        wt1.reshape(128, 480),
        wtg.reshape(128, 360),
        wth.reshape(128, 720),
        wid_t, wg2_t, wh2_t,
        fold_lhs(wq, lnx1_g, lnx1_b), fold_lhs(wk, lnx2_g, lnx2_b),
        fold_lhs(wv, lnx2_g, lnx2_b, scale=GATE_SCALE / 2),
        indic, indicT, indicg, indicgT, vecs,
        np.eye(128, dtype=f)], axis=1).astype(f)
    shared = dict(wpack=wpack, onesrow=np.ones(4096, f))
    in_maps = []
    for core in range(8):
        b, h = divmod(core, 2)
        in_maps.append(dict(
            x1b=band_pack(np.asarray(x1)[b].astype(f), C1, shift_dup=True)[h],
            x2b=band_pack(np.asarray(x2)[b].astype(f), K)[h],
            **shared))
    return in_maps


def kernel(**inputs):
    global _BUILT
    if _BUILT is None:
        _BUILT = _build()
    in_maps = _prep_inputs(**inputs)
    last_err = None
    for _ in range(3):
        try:
            res = run_bass_kernel_spmd(_BUILT, in_maps, list(range(8))).results
            break
        except Exception as e:  # transient axon worker hangups
            last_err = e
    else:
        raise last_err
    pred = np.stack([res[2 * b]["pred"] for b in range(B)]).astype(np.float32)
    xmin = np.stack([res[2 * b]["xminv"] for b in range(B)]).astype(np.float32)
    return pred, xmin


# revision 16
# speedup vs baseline: 1.3332x; 1.2479x over previous
"""Trainium2 Bass kernel for nn_Expert_layer2 (dense per-sample HWxHW attention block).

Sharding: 8 cores = 4 samples x 2 query-halves. Core (b, h) sees sample b
VERTICALLY FLIPPED when h=1 (host flips rows and conv-kernel dy), so every
core uniformly owns band rows 0..31 of its (possibly flipped) image, band
j=0 = own half, j=1 = other half. Each core computes conv/GN/LN projections,
attention for its 2048 queries over all 4096 keys, then exchanges ONE halo
row (band row 31) with its pair partner inside the qc loop, computes the
conv head on its own half only, pair-AllReduces the head GN stats (tiny),
and writes its half of pred/xmin. Host re-assembles (un-flipping h=1).

v3 notes:
- x1 conv packs 2 taps per matmul (host duplicates x1 column-shifted into
  partitions 64:128): 48 matmuls instead of 72.
- GN channel stats via bn_stats/bn_aggr (DVE) instead of square+accum (Act).
- no full-tile memsets: matmuls contract only the live partition rows.
- act tables: Sqrt region -> Silu/Tanh region -> Ln/Exp region (attention,
  rsqrt via exp(-0.5*ln(v+eps))) -> Silu/Tanh region (head). 4 loads total.
- gate sigmoid folded into tanh (+ lhsv pre-scaled by GATE_SCALE/2), pred
  sigmoid via tanh.
- qc order [3,0,1,2]: halo row ready after first chunk, AllGather hidden
  under remaining attention; head conv chunks interleaved into qc shadow.
"""

from contextlib import ExitStack

import numpy as np
import concourse.bass as bass
from concourse import bacc
import concourse.tile as tile
import concourse.mybir as mybir
import concourse.bass_isa as bass_isa
from concourse.bass_utils import run_bass_kernel_spmd

F32 = mybir.dt.float32
F32R = mybir.dt.float32r
AF = mybir.ActivationFunctionType
ALU = mybir.AluOpType

B, C1, K, H, W = 4, 64, 80, 64, 64
HW = H * W
HALF = HW // 2
EPS = 1e-5
GATE_SCALE = 0.1
ISQK = float(1.0 / np.sqrt(np.float32(K)))

_BUILT = None


def _build():
    nc = bacc.Bacc("TRN2", target_bir_lowering=False, num_devices=8)

    dd = {}
    dd["x1b"] = nc.dram_tensor("x1b", [128, 2, 34, 66], F32, kind="ExternalInput")
    dd["x2b"] = nc.dram_tensor("x2b", [128, 2, 34, 66], F32, kind="ExternalInput")
    dd["wpack"] = nc.dram_tensor("wpack", [128, 2231], F32, kind="ExternalInput")
    dd["onesrow"] = nc.dram_tensor("onesrow", [4096], F32, kind="ExternalInput")
    dd["pred_o"] = nc.dram_tensor("pred", [2048], F32, kind="ExternalOutput")
    dd["xmin_o"] = nc.dram_tensor("xminv", [2048], F32, kind="ExternalOutput")
    dd["ch_in"] = nc.dram_tensor("ch_in", [80, 64], F32)
    dd["ch_out"] = nc.dram_tensor("ch_out", [2, 80, 64], F32)
    dd["cs_in"] = nc.dram_tensor("cs_in", [80, 2], F32)
    dd["cs_out"] = nc.dram_tensor("cs_out", [2, 80, 2], F32)

    with tile.TileContext(nc) as tc:
        _body(nc, tc, dd)
    nc.finalize()
    return nc


def _body(nc, tc, dd):
    onesrow = dd["onesrow"]
    RG = [[0, 1], [2, 3], [4, 5], [6, 7]]
    ctx = ExitStack()
    with ctx:
        pw = ctx.enter_context(tc.tile_pool(name="pw", bufs=1))
        prow = ctx.enter_context(tc.tile_pool(name="prow", bufs=1))
        pscr = ctx.enter_context(tc.tile_pool(name="pscr", bufs=2))
        pscr1 = ctx.enter_context(tc.tile_pool(name="pscr1", bufs=1))
        pror = ctx.enter_context(tc.tile_pool(name="pror", bufs=1))
        psmA = ctx.enter_context(tc.tile_pool(name="psmA", bufs=4, space="PSUM"))
        psmB = ctx.enter_context(tc.tile_pool(name="psmB", bufs=2, space="PSUM"))

        # ------------ persistent weights (one packed DMA) ------------
        wpk = pw.tile([128, 2231], F32R, tag="wpk")
        nc.sync.dma_start(out=wpk[:, 0:744], in_=dd["wpack"][:, 0:744].bitcast(F32R))
        nc.gpsimd.dma_start(out=wpk[:, 744:1488], in_=dd["wpack"][:, 744:1488].bitcast(F32R))
        nc.sync.dma_start(out=wpk[:, 1488:2231], in_=dd["wpack"][:, 1488:2231].bitcast(F32R))
        off = [0]

        def _wslice(n):
            a = wpk[:, off[0]:off[0] + n]
            off[0] += n
            return a

        wt1 = _wslice(480).rearrange("p (t f) -> p t f", t=6)
        wtg = _wslice(360).rearrange("p (t f) -> p t f", t=9)
        wth = _wslice(720).rearrange("p (t f) -> p t f", t=9)
        ws = {}
        for nm, fd in (("wid", 80), ("wg2", 80), ("wh2", 1), ("lhsq", 80), ("lhsk", 80),
                       ("lhsv", 80), ("indic", 5), ("indicT", 80), ("indicg", 5),
                       ("indicgT", 40)):
            ws[nm] = _wslice(fd)
        vec_sb = _wslice(12).bitcast(F32)
        ident_sb = _wslice(128)
        assert off[0] == 2231
        ones_col = pw.tile([128, 1], F32R, tag="onescol")
        nc.sync.dma_start(out=ones_col[:], in_=onesrow[0:128, None].bitcast(F32R))
        eps_col = pw.tile([128, 1], F32, tag="epscol")
        nc.vector.memset(eps_col[:], EPS)

        gn1_g, gn1_b = vec_sb[0:80, 0:1], vec_sb[0:80, 1:2]
        gnid_g, gnid_b = vec_sb[0:80, 2:3], vec_sb[0:80, 3:4]
        gng_g, gng_b = vec_sb[0:40, 4:5], vec_sb[0:40, 5:6]
        gnh_g, gnh_b = vec_sb[0:80, 6:7], vec_sb[0:80, 7:8]
        lno_g, lno_b = vec_sb[0:80, 8:9], vec_sb[0:80, 9:10]
        bg2h_ap = vec_sb[0:80, 10:11]
        bh2h_ap = vec_sb[0:1, 11:12]

        # ============ helpers ============
        def ts_chunk(dn, c, src):
            """Per-chunk token sums/sumsq -> dn[:, c*512:...]. src f32r [80,8,64]."""
            sqs = pscr.tile([1, 1024], F32, tag="csq")
            ps_s = psmA.tile([1, 512], F32, tag="cps")
            nc.tensor.matmul(ps_s[:], ones_col[0:80], src, start=True, stop=True)
            nc.vector.tensor_copy(sqs[:, 0:512], ps_s[:])
            sq = pscr.tile([80, 8, 64], F32R, tag="stq")
            nc.scalar.activation(sq[:], src.bitcast(F32), AF.Square)
            ps_q = psmA.tile([1, 512], F32, tag="cps")
            nc.tensor.matmul(ps_q[:], ones_col[0:80], sq[:], start=True, stop=True)
            nc.scalar.copy(sqs[:, 512:1024], ps_q[:])
            nc.sync.dma_start(out=dn[:, c * 512:(c + 1) * 512], in_=sqs[:])

        def ts_finish(nm, dn, ntok, mr_target_ap, lnexp):
            """Token LN stats math; writes r*m row to mr_target_ap, returns r_row."""
            up = nc.dram_tensor(f"up_{nm}", [2, ntok], F32)
            ncol = ntok // 128
            st = prow.tile([128, 2, ncol], F32, tag=f"stt_{nm}")
            nc.sync.dma_start(out=st[:], in_=dn.rearrange("r (p c) -> p r c", p=128))
            m = prow.tile([128, ncol], F32, tag=f"m_{nm}")
            nc.vector.tensor_scalar(m[:], st[:, 0, :], 1.0 / K, None, ALU.mult)
            msq = prow.tile([128, ncol], F32, tag=f"ms_{nm}")
            nc.vector.tensor_tensor(msq[:], m[:], m[:], ALU.mult)
            var = prow.tile([128, ncol], F32, tag=f"v_{nm}")
            nc.vector.tensor_scalar(var[:], st[:, 1, :], 1.0 / K, None, ALU.mult)
            nc.vector.tensor_tensor(var[:], var[:], msq[:], ALU.subtract)
            rup = prow.tile([128, 2, ncol], F32, tag=f"ru_{nm}")
            if lnexp:
                nc.scalar.activation(var[:], var[:], AF.Ln, bias=eps_col[:])
                nc.scalar.activation(rup[:, 0, :], var[:], AF.Exp, scale=-0.5)
            else:
                nc.scalar.activation(var[:], var[:], AF.Sqrt, bias=eps_col[:])
                nc.vector.reciprocal(rup[:, 0, :], var[:])
            nc.vector.tensor_tensor(rup[:, 1, :], rup[:, 0, :], m[:], ALU.mult)
            nc.sync.dma_start(out=up.rearrange("r (p c) -> p r c", p=128), in_=rup[:])
            r_row = pror.tile([1, 4096], F32, tag="rrow", name="r_row")[:, 0:ntok]
            nc.sync.dma_start(out=r_row, in_=up[0, None, :])
            nc.sync.dma_start(out=mr_target_ap, in_=up[1, None, :].bitcast(F32R))
            return r_row

        def group_params(nm, s_ch, nch, ind_sb, indT_sb, gamma, beta, inv_n, lnexp):
            """s_ch [nch,2] f32r = per-channel (A, B); group mean = sum(A)*inv_n,
            group var = sum(B)*inv_n - mean^2. Returns (a, c) per-channel."""
            ps_g = psmA.tile([5, 2], F32, tag="cps")
            nc.tensor.matmul(ps_g[:], ind_sb[0:nch, :], s_ch[:], start=True, stop=True)
            gst = prow.tile([5, 2], F32R, tag=f"gst_{nm}")
            nc.vector.tensor_scalar(gst[:, 0:1], ps_g[:, 0:1], inv_n, None, ALU.mult)
            msq = prow.tile([5, 1], F32, tag=f"msq_{nm}")
            nc.vector.tensor_tensor(msq[:], gst[:, 0:1].bitcast(F32),
                                    gst[:, 0:1].bitcast(F32), ALU.mult)
            vr = prow.tile([5, 1], F32, tag=f"vr_{nm}")
            nc.vector.tensor_scalar(vr[:], ps_g[:, 1:2], inv_n, None, ALU.mult)
            nc.vector.tensor_tensor(vr[:], vr[:], msq[:], ALU.subtract)
            if lnexp:
                nc.scalar.activation(vr[:], vr[:], AF.Ln, bias=eps_col[0:5, :])
                nc.scalar.activation(gst[:, 1:2], vr[:], AF.Exp, scale=-0.5)
            else:
                nc.scalar.activation(vr[:], vr[:], AF.Sqrt, bias=eps_col[0:5, :])
                with nc.allow_low_precision(reason="f32r == f32 bits"):
                    nc.vector.reciprocal(gst[:, 1:2], vr[:])
            ps_bc = psmA.tile([nch, 2], F32, tag="cps")
            nc.tensor.matmul(ps_bc[:], indT_sb[0:5, 0:nch], gst[:], start=True, stop=True)
            a = prow.tile([nch, 1], F32, tag=f"a_{nm}")
            c = prow.tile([nch, 1], F32, tag=f"c_{nm}")
            nc.vector.tensor_tensor(a[:], gamma, ps_bc[:, 1:2], ALU.mult)
            nc.vector.tensor_tensor(c[:], ps_bc[:, 0:1], a[:], ALU.mult)
            nc.vector.tensor_tensor(c[:], beta, c[:], ALU.subtract)
            return a, c

        def bn_to_sums(nm, bst, nch, nchunk):
            """bn_aggr chunks -> s_ch [nch,2] f32r = per-channel (mean, E[x^2])."""
            mv = prow.tile([nch, 2], F32, tag=f"mv_{nm}")
            nc.vector.bn_aggr(mv[:], bst[:, 0:nchunk, :])
            s_ch = prow.tile([nch, 2], F32R, tag=f"sch_{nm}")
            nc.vector.tensor_copy(s_ch[:, 0:1], mv[:, 0:1])
            mm = prow.tile([nch, 1], F32, tag=f"mm_{nm}")
            nc.vector.tensor_tensor(mm[:], mv[:, 0:1], mv[:, 0:1], ALU.mult)
            nc.vector.tensor_tensor(s_ch[:, 1:2], mv[:, 1:2], mm[:], ALU.add)
            return s_ch

        # ============ phase A ============
        p2b = ctx.enter_context(tc.tile_pool(name="p2b", bufs=1))
        ctx_mid = ExitStack()
        pin2 = ctx_mid.enter_context(tc.tile_pool(name="pin2", bufs=1))
        pmid = ctx_mid.enter_context(tc.tile_pool(name="pmid", bufs=1))

        # x2 input first (independent SBUF region, loads during x1 conv)
        x2pad = pin2.tile([128, 2, 34, 66], F32R, tag="x2pad")
        nc.gpsimd.dma_start(out=x2pad[:, 0], in_=dd["x2b"][:, 0].bitcast(F32R))
        nc.gpsimd.dma_start(out=x2pad[:, 1], in_=dd["x2b"][:, 1].bitcast(F32R))

        def x2win(c8):
            j, c4 = divmod(c8, 4)
            return x2pad[0:80, j, 1 + c4 * 8: 9 + c4 * 8, 1:65]

        x1cp = pmid.tile([80, HALF], F32R, tag="x1cp")
        bst1 = prow.tile([80, 8, 6], F32, tag="bst1")
        xh2 = pmid.tile([82, 4096], F32R, tag="xh2")
        dn2 = nc.dram_tensor("dn_x2", [2, 4096], F32)

        # x1 conv (paired taps) interleaved with x2 token-stat chunks
        with tc.tile_pool(name="pin1", bufs=1) as pin1:
            x1pad = pin1.tile([128, 2, 34, 66], F32R, tag="x1pad")
            nc.sync.dma_start(out=x1pad[:, 0], in_=dd["x1b"][:, 0].bitcast(F32R))
            nc.sync.dma_start(out=x1pad[:, 1], in_=dd["x1b"][:, 1].bitcast(F32R))
            for c8 in range(8):
                j, c4 = divmod(c8, 4)
                ps = psmA.tile([80, 512], F32, tag="cps")
                for dy in range(3):
                    r0 = c4 * 8 + dy
                    nc.tensor.matmul(ps[:], wt1[:, 2 * dy, :],
                                     x1pad[:, j, r0:r0 + 8, 0:64],
                                     start=(dy == 0), stop=False)
                    nc.tensor.matmul(ps[:], wt1[0:64, 2 * dy + 1, :],
                                     x1pad[0:64, j, r0:r0 + 8, 2:66],
                                     start=False, stop=(dy == 2))
                if c8 < 4:
                    nc.scalar.activation(x1cp[:, c8 * 512:(c8 + 1) * 512], ps[:], AF.Copy)
                nc.vector.bn_stats(bst1[:, c8, :], ps[:])
                ts_chunk(dn2, c8, x2win(c8))

        s1 = bn_to_sums("1", bst1, 80, 8)
        a1, c1 = group_params("1", s1, 80, ws["indic"], ws["indicT"],
                              gn1_g, gn1_b, 1.0 / 16, lnexp=False)

        # x2 token LN stats -> xh2 [82, 4096]
        r2_row = ts_finish("x2", dn2, 4096, xh2[80:81, :], lnexp=False)
        nc.sync.dma_start(out=xh2[81:82, :], in_=onesrow[None, :].bitcast(F32R))

        # gate conv3x3 + idc conv1x1 (chunk-interleaved)
        gcp = pmid.tile([40, 4096], F32R, tag="gcp")
        bstg = prow.tile([40, 8, 6], F32, tag="bstg")
        idc = p2b.tile([80, HALF], F32, tag="idc")
        bstid = prow.tile([80, 8, 6], F32, tag="bstid")
        for c8 in range(8):
            j, c4 = divmod(c8, 4)
            ps = psmA.tile([40, 512], F32, tag="cps")
            for tap in range(9):
                dy, dx = divmod(tap, 3)
                nc.tensor.matmul(ps[:], wtg[0:80, tap, :],
                                 x2pad[0:80, j, c4 * 8 + dy: c4 * 8 + dy + 8, dx:dx + 64],
                                 start=(tap == 0), stop=(tap == 8))
            nc.scalar.activation(gcp[:, c8 * 512:(c8 + 1) * 512], ps[:], AF.Copy)
            nc.vector.bn_stats(bstg[:, c8, :], ps[:])
            ps_id = psmA.tile([80, 512], F32, tag="cps")
            nc.tensor.matmul(ps_id[:], ws["wid"][0:80, :], x2win(c8), start=True, stop=True)
            if c8 < 4:
                nc.scalar.activation(idc[:, c8 * 512:(c8 + 1) * 512], ps_id[:], AF.Copy)
            nc.vector.bn_stats(bstid[:, c8, :], ps_id[:])
        sg = bn_to_sums("g", bstg, 40, 8)
        ag, cg = group_params("g", sg, 40, ws["indicg"], ws["indicgT"],
                              gng_g, gng_b, 1.0 / 8, lnexp=False)
        sid = bn_to_sums("id", bstid, 80, 8)
        aid, cid = group_params("id", sid, 80, ws["indic"], ws["indicT"],
                                gnid_g, gnid_b, 1.0 / 16, lnexp=False)

        # ---- silu/tanh act-table region ----
        nc.scalar.activation(x1cp[:, :], x1cp[:, :].bitcast(F32), AF.Silu,
                             bias=c1[:], scale=a1[:])
        nc.scalar.activation(gcp[:, :], gcp[:, :].bitcast(F32), AF.Silu,
                             bias=cg[:], scale=ag[:])
        nc.vector.tensor_scalar(idc[:], idc[:], aid[:], cid[:], ALU.mult, ALU.add)

        # xh2 rows 0:80 = x2 * r (token LN; gamma/beta folded into lhs weights)
        for c in range(8):
            sl = slice(c * 512, (c + 1) * 512)
            bc = pscr.tile([80, 512], F32, tag="bscr", name="bc")
            nc.gpsimd.partition_broadcast(bc[:], r2_row[0:1, sl])
            nc.vector.tensor_tensor(xh2[0:80, sl].rearrange("p (a b) -> p a b", a=8),
                                    x2win(c).bitcast(F32),
                                    bc[:].rearrange("p (a b) -> p a b", a=8), ALU.mult)

        # k, v projections (LN folded into lhs; lhsv pre-scaled GATE_SCALE/2)
        k_sb = p2b.tile([80, 4096], F32R, tag="k_sb")
        v_sb = pmid.tile([80, 4096], F32R, tag="v_sb")
        for c in range(8):
            sl = slice(c * 512, (c + 1) * 512)
            ps_k = psmA.tile([80, 512], F32, tag="cps")
            nc.tensor.matmul(ps_k[:], ws["lhsk"][0:82, :], xh2[:, sl], start=True, stop=True)
            nc.vector.tensor_copy(k_sb[:, sl], ps_k[:])
            ps_v = psmA.tile([80, 512], F32, tag="cps")
            nc.tensor.matmul(ps_v[:], ws["lhsv"][0:82, :], xh2[:, sl], start=True, stop=True)
            nc.vector.tensor_copy(v_sb[:, sl], ps_v[:])

        # gate 1x1 via tanh trick: v *= (tanh((wg2@g+bg2)/2) + 21)
        dn1 = nc.dram_tensor("dn_x1", [2, HALF], F32)
        for c in range(8):
            sl = slice(c * 512, (c + 1) * 512)
            ps_g2 = psmA.tile([80, 512], F32, tag="cps")
            nc.tensor.matmul(ps_g2[:], ws["wg2"][0:40, :], gcp[:, sl],
                             start=True, stop=True)
            tg = pscr.tile([80, 512], F32, tag="bscr")
            nc.scalar.activation(tg[:], ps_g2[:], AF.Tanh, bias=bg2h_ap, scale=0.5)
            nc.vector.tensor_scalar(tg[:], tg[:], 21.0, None, ALU.add)
            nc.vector.tensor_tensor(v_sb[:, sl], v_sb[:, sl].bitcast(F32), tg[:], ALU.mult)
            # x1 token-stat chunks ride along (squares are in-table)
            if c < 4:
                ts_chunk(dn1, c, x1cp[:, sl].rearrange("p (a b) -> p a b", a=8))

        # vT: token-major gated v in cols 0..79, ones in col 96 (denominator row)
        vT = p2b.tile([128, 32, 97], F32R, tag="vT")
        nc.vector.memset(vT[:, :, 96:97].bitcast(F32), 1.0)
        for kc in range(32):
            ps_t = psmA.tile([128, 80], F32R, tag="cps")
            nc.tensor.transpose(ps_t[:], v_sb[:, kc * 128:(kc + 1) * 128],
                                ident_sb[0:80, 0:80])
            nc.vector.tensor_copy(vT[:, kc, 0:80], ps_t[:])

        # ---- ln/exp act-table region: x1 token LN -> xh1 -> q, then attention
        xh1 = pmid.tile([82, 4096], F32R, tag="v_sb", name="xh1")[:, 0:HALF]
        r1_row = ts_finish("x1", dn1, HALF, xh1[80:81, :], lnexp=True)
        nc.sync.dma_start(out=xh1[81:82, :], in_=onesrow[None, 0:HALF].bitcast(F32R))
        q_sb = p2b.tile([80, HALF], F32R, tag="q_sb")
        for c in range(4):
            sl = slice(c * 512, (c + 1) * 512)
            bc = pscr.tile([80, 512], F32, tag="bscr", name="bc")
            nc.gpsimd.partition_broadcast(bc[:], r1_row[0:1, sl])
            nc.vector.tensor_tensor(xh1[0:80, sl], x1cp[:, sl].bitcast(F32),
                                    bc[:], ALU.mult)
            ps_q = psmA.tile([80, 512], F32, tag="cps")
            nc.tensor.matmul(ps_q[:], ws["lhsq"][0:82, :], xh1[:, sl], start=True, stop=True)
            nc.vector.tensor_copy(q_sb[:, sl], ps_q[:])

        ctx_mid.close()

        # ============ attention + interleaved head conv ============
        ph = ctx.enter_context(tc.tile_pool(name="ph", bufs=1))
        yres = p2b.tile([80, HALF], F32R, tag="yres")
        ypad = ph.tile([80, 34, 66], F32R, tag="ypad")
        nc.vector.memset(ypad[:, :, 0:1].bitcast(F32), 0.0)
        nc.vector.memset(ypad[:, :, 65:66].bitcast(F32), 0.0)
        nc.vector.memset(ypad[:, 0:1, 1:65].bitcast(F32), 0.0)
        hcp = ph.tile([80, HALF], F32R, tag="hcp")
        bsth = prow.tile([80, 4, 6], F32, tag="bsth")
        hld = ph.tile([80, 2, 64], F32, tag="hld")

        def head_chunk(c4):
            ps = psmA.tile([80, 512], F32, tag="cps")
            for tap in range(9):
                dy, dx = divmod(tap, 3)
                nc.tensor.matmul(ps[:], wth[0:80, tap, :],
                                 ypad[:, c4 * 8 + dy: c4 * 8 + dy + 8, dx:dx + 64],
                                 start=(tap == 0), stop=(tap == 8))
            nc.scalar.activation(hcp[:, c4 * 512:(c4 + 1) * 512], ps[:], AF.Copy)
            nc.vector.bn_stats(bsth[:, c4, :], ps[:])

        for it, qc in enumerate((3, 0, 1, 2)):
            qsl = slice(qc * 512, (qc + 1) * 512)
            ps_y = psmA.tile([97, 512], F32, tag="cps")
            for k2 in range(16):
                ps_s = psmB.tile([128, 1024], F32, tag="scps", name="ps_s")
                nc.tensor.matmul(ps_s[:, 0:512], k_sb[:, (2 * k2) * 128:(2 * k2 + 1) * 128],
                                 q_sb[:, qsl], start=True, stop=True)
                nc.tensor.matmul(ps_s[:, 512:1024], k_sb[:, (2 * k2 + 1) * 128:(2 * k2 + 2) * 128],
                                 q_sb[:, qsl], start=True, stop=True)
                e_sb = pscr.tile([128, 1024], F32R, tag="e_sb")
                nc.scalar.activation(e_sb[:], ps_s[:], AF.Exp, scale=ISQK)
                nc.tensor.matmul(ps_y[:], vT[:, 2 * k2, :], e_sb[:, 0:512],
                                 start=(k2 == 0), stop=False)
                nc.tensor.matmul(ps_y[:], vT[:, 2 * k2 + 1, :], e_sb[:, 512:1024],
                                 start=False, stop=(k2 == 15))
            rec = pscr.tile([1, 512], F32, tag="bscr")
            nc.vector.reciprocal(rec[:], ps_y[96:97, :])
            rb = pscr.tile([80, 512], F32, tag="bscr")
            nc.gpsimd.partition_broadcast(rb[:], rec[:])
            nc.vector.tensor_tensor(yres[:, qsl], ps_y[0:80, :], rb[:], ALU.mult)
            nc.vector.tensor_tensor(yres[:, qsl], yres[:, qsl].bitcast(F32),
                                    idc[:, qsl], ALU.add)

            # out-LN on this 512-token block; rsqrt via exp(-0.5*ln(v+eps))
            src = yres[:, qsl].rearrange("p (a b) -> p a b", a=8)
            ps_s = psmA.tile([1, 512], F32, tag="cps", name="ps_s_o")
            nc.tensor.matmul(ps_s[:], ones_col[0:80], src, start=True, stop=True)
            sqo = pscr.tile([80, 8, 64], F32R, tag="stq", name="sqo")
            nc.scalar.activation(sqo[:], src.bitcast(F32), AF.Square)
            ps_q = psmA.tile([1, 512], F32, tag="cps", name="ps_q_o")
            nc.tensor.matmul(ps_q[:], ones_col[0:80], sqo[:], start=True, stop=True)
            t_m = pscr1.tile([1, 512], F32, tag="tm")
            nc.vector.tensor_scalar(t_m[:], ps_s[:], 1.0 / K, None, ALU.mult)
            t_n = pscr1.tile([1, 512], F32, tag="tn")
            nc.vector.tensor_tensor(t_n[:], t_m[:], t_m[:], ALU.mult)
            t_r = pscr1.tile([1, 512], F32, tag="tr")
            nc.vector.tensor_scalar(t_r[:], ps_q[:], 1.0 / K, None, ALU.mult)
            nc.vector.tensor_tensor(t_r[:], t_r[:], t_n[:], ALU.subtract)
            nc.scalar.activation(t_r[:], t_r[:], AF.Ln, bias=eps_col[0:1, :])
            nc.scalar.activation(t_r[:], t_r[:], AF.Exp, scale=-0.5)
            nc.vector.tensor_tensor(t_n[:], t_r[:], t_m[:], ALU.mult)
            nc.vector.tensor_scalar(t_n[:], t_n[:], -1.0, None, ALU.mult)
            bc = pscr.tile([80, 512], F32, tag="bscr", name="bc")
            nc.gpsimd.partition_broadcast(bc[:], t_r[:])
            mbc = pscr.tile([80, 512], F32, tag="bscr", name="mbc")
            nc.gpsimd.partition_broadcast(mbc[:], t_n[:])
            yl = pscr.tile([80, 512], F32, tag="csq")
            nc.vector.tensor_tensor(yl[:], yres[:, qsl].bitcast(F32), bc[:], ALU.mult)
            nc.vector.tensor_tensor(yl[:], yl[:], mbc[:], ALU.add)
            nc.vector.tensor_scalar(ypad[:, 1 + 8 * qc: 9 + 8 * qc, 1:65],
                                    yl[:].rearrange("p (a b) -> p a b", a=8),
                                    lno_g, lno_b, ALU.mult, ALU.add)

            if it == 0:
                # band row 31 ready -> pair halo exchange (hidden under qc loop)
                nc.sync.dma_start(out=dd["ch_in"][:].bitcast(F32R),
                                  in_=ypad[:, 32, 1:65])
                nc.gpsimd.collective_compute(
                    "AllGather", ALU.bypass, replica_groups=RG,
                    ins=[dd["ch_in"][:]], outs=[dd["ch_out"][:]])
            elif it == 1:
                nc.sync.dma_start(out=hld[:], in_=dd["ch_out"].rearrange("s p c -> p s c"))
            elif it == 2:
                head_chunk(0)
            elif it == 3:
                # halo row = (slot0 + slot1) - own row 31
                hsum = pscr1.tile([80, 64], F32, tag="hsum")
                nc.vector.tensor_tensor(hsum[:], hld[:, 0, :], hld[:, 1, :], ALU.add)
                nc.vector.tensor_tensor(ypad[:, 33, 1:65], hsum[:],
                                        ypad[:, 32, 1:65].bitcast(F32), ALU.subtract)

        for c4 in range(1, 4):
            head_chunk(c4)

        # head GN stats: pair-exchange per-channel sums (tiny)
        mvh = prow.tile([80, 2], F32, tag="mvh")
        nc.vector.bn_aggr(mvh[:], bsth[:])
        s_half = prow.tile([80, 2], F32, tag="s_half")
        nc.vector.tensor_scalar(s_half[:, 0:1], mvh[:, 0:1], float(HALF), None, ALU.mult)
        e2h = prow.tile([80, 1], F32, tag="e2h")
        nc.vector.tensor_tensor(e2h[:], mvh[:, 0:1], mvh[:, 0:1], ALU.mult)
        nc.vector.tensor_tensor(e2h[:], mvh[:, 1:2], e2h[:], ALU.add)
        nc.vector.tensor_scalar(s_half[:, 1:2], e2h[:], float(HALF), None, ALU.mult)
        nc.sync.dma_start(out=dd["cs_in"][:], in_=s_half[:])
        nc.gpsimd.collective_compute(
            "AllGather", ALU.bypass, replica_groups=RG,
            ins=[dd["cs_in"][:]], outs=[dd["cs_out"][:]])

        # xmin (own half; fills the stats-collective window)
        for c2 in range(2):
            xw = pscr.tile([80, 16, 64], F32, tag="csq")
            nc.sync.dma_start(out=xw[:], in_=dd["x2b"][0:80, 0, 1 + 16 * c2: 17 + 16 * c2, 1:65])
            ng = pscr.tile([80, 1024], F32, tag="csq")
            nc.vector.tensor_scalar(ng[:].rearrange("p (a b) -> p a b", a=16),
                                    xw[:], -1.0, None, ALU.mult)
            ar = pscr.tile([80, 1024], F32, tag="csq")
            nc.gpsimd.partition_all_reduce(ar[:], ng[:], channels=80,
                                           reduce_op=bass_isa.ReduceOp.max)
            xm = pror.tile([1, 4096], F32, tag="rrow", name="xm")
            nc.vector.tensor_scalar(xm[:, 0:1024], ar[0:1, :], -1.0, None, ALU.mult)
            nc.sync.dma_start(out=dd["xmin_o"][None, c2 * 1024:(c2 + 1) * 1024],
                              in_=xm[:, 0:1024])

        sld = ph.tile([80, 2, 2], F32, tag="sld")
        nc.sync.dma_start(out=sld[:], in_=dd["cs_out"].rearrange("s p c -> p s c"))
        s_full = prow.tile([80, 2], F32R, tag="s_full")
        nc.vector.tensor_tensor(s_full[:], sld[:, 0, :], sld[:, 1, :], ALU.add)
        ahd, chd = group_params("h", s_full, 80, ws["indic"], ws["indicT"],
                                gnh_g, gnh_b, 1.0 / (16 * HW), lnexp=True)

        # ---- final silu/tanh region ----
        nc.scalar.activation(hcp[:, :], hcp[:, :].bitcast(F32), AF.Silu,
                             bias=chd[:], scale=ahd[:])
        pr = pror.tile([1, 4096], F32, tag="rrow", name="pr")
        for c in range(4):
            sl = slice(c * 512, (c + 1) * 512)
            ps_p = psmA.tile([1, 512], F32, tag="cps")
            nc.tensor.matmul(ps_p[:], ws["wh2"][0:80, :], hcp[:, sl], start=True, stop=True)
            nc.scalar.activation(pr[:, sl], ps_p[:], AF.Tanh, bias=bh2h_ap, scale=0.5)
            nc.vector.tensor_scalar(pr[:, sl], pr[:, sl], 0.5, 0.5, ALU.mult, ALU.add)
        nc.sync.dma_start(out=dd["pred_o"][None, :], in_=pr[:, 0:HALF])


def _prep_inputs(x1, x2, w_p1, gn1_g, gn1_b, w_id, gnid_g, gnid_b, wq, wk, wv,
                 wg1, gng_g, gng_b, wg2, bg2, lnx1_g, lnx1_b, lnx2_g, lnx2_b,
                 lno_g, lno_b, wh1, gnh_g, gnh_b, wh2, bh2):
    f = np.float32

    def band_pack(x, C, shift_dup=False):
        # uniform core view: band j=0 = rows 0..31 (+halo), j=1 = rows 32..63
        arr = np.zeros((128, 2, 34, 66), f)
        for j in range(2):
            g0 = 32 * j - 1
            lo, hi = max(0, g0), min(64, g0 + 34)
            arr[:C, j, lo - g0: hi - g0, 1:65] = x[:, lo:hi, :]
        if shift_dup:
            arr[64:128, :, :, 0:65] = arr[0:64, :, :, 1:66]
        return arr

    def fold_lhs(wmat, lng, lnb, scale=1.0):
        wmat = np.asarray(wmat).astype(f) * f(scale)
        wp = wmat * np.asarray(lng).astype(f)[None, :]
        lhs = np.zeros((128, 80), f)
        lhs[0:80, :] = wp.T
        lhs[80, :] = -wp.sum(axis=1)
        lhs[81, :] = wmat @ np.asarray(lnb).astype(f)
        return lhs

    wid_t = np.zeros((128, 80), f); wid_t[0:80, :] = np.asarray(w_id)[:, :, 0, 0].T
    wg2_t = np.zeros((128, 80), f); wg2_t[0:40, :] = np.asarray(wg2)[:, :, 0, 0].T
    wh2_t = np.zeros((128, 1), f); wh2_t[0:80, 0] = np.asarray(wh2)[0, :, 0, 0]
    indic = np.zeros((128, 5), f); indicT = np.zeros((128, 80), f)
    for ch in range(80):
        indic[ch, ch // 16] = 1.0
        indicT[ch // 16, ch] = 1.0
    indicg = np.zeros((128, 5), f); indicgT = np.zeros((128, 40), f)
    for ch in range(40):
        indicg[ch, ch // 8] = 1.0
        indicgT[ch // 8, ch] = 1.0
    vecs = np.zeros((128, 12), f)
    for col, v, n in ((0, gn1_g, 80), (1, gn1_b, 80), (2, gnid_g, 80), (3, gnid_b, 80),
                     (4, gng_g, 40), (5, gng_b, 40), (6, gnh_g, 80), (7, gnh_b, 80),
                     (8, lno_g, 80), (9, lno_b, 80),
                     (10, 0.5 * np.asarray(bg2), 80), (11, 0.5 * np.asarray(bh2), 1)):
        vecs[0:n, col] = np.asarray(v).astype(f)

    def make_wpack(wp1, wg1a, wh1a):
        wt1 = np.zeros((128, 6, 80), f)
        for dy in range(3):
            wt1[0:64, 2 * dy, :] = wp1[:, :, dy, 0].T
            wt1[64:128, 2 * dy, :] = wp1[:, :, dy, 1].T
            wt1[0:64, 2 * dy + 1, :] = wp1[:, :, dy, 2].T
        wtg = np.zeros((128, 9, 40), f)
        for t in range(9):
            dy, dx = divmod(t, 3)
            wtg[0:80, t, :] = wg1a[:, :, dy, dx].T
        wth = np.zeros((128, 9, 80), f)
        for t in range(9):
            dy, dx = divmod(t, 3)
            wth[0:80, t, :] = wh1a[:, :, dy, dx].T
        return np.concatenate([
            wt1.reshape(128, 480), wtg.reshape(128, 360), wth.reshape(128, 720),
            wid_t, wg2_t, wh2_t,
            fold_lhs(wq, lnx1_g, lnx1_b), fold_lhs(wk, lnx2_g, lnx2_b),
            fold_lhs(wv, lnx2_g, lnx2_b, scale=GATE_SCALE / 2),
            indic, indicT, indicg, indicgT, vecs,
            np.eye(128, dtype=f)], axis=1).astype(f)

    wp1 = np.asarray(w_p1).astype(f)
    wg1a = np.asarray(wg1).astype(f)
    wh1a = np.asarray(wh1).astype(f)
    wpack_h = {0: make_wpack(wp1, wg1a, wh1a),
               1: make_wpack(wp1[:, :, ::-1, :], wg1a[:, :, ::-1, :],
                             wh1a[:, :, ::-1, :])}
    ones = np.ones(4096, f)
    in_maps = []
    for core in range(8):
        b, h = divmod(core, 2)
        xa = np.asarray(x1)[b].astype(f)
        xb = np.asarray(x2)[b].astype(f)
        if h == 1:
            xa, xb = xa[:, ::-1, :], xb[:, ::-1, :]
        in_maps.append(dict(
            x1b=band_pack(xa, C1, shift_dup=True),
            x2b=band_pack(xb, K),
            wpack=wpack_h[h], onesrow=ones))
    return in_maps


def kernel(**inputs):
    global _BUILT
    if _BUILT is None:
        _BUILT = _build()
    in_maps = _prep_inputs(**inputs)
    last_err = None
    for _ in range(3):
        try:
            res = run_bass_kernel_spmd(_BUILT, in_maps, list(range(8))).results
            break
        except Exception as e:  # transient axon worker hangups
            last_err = e
    else:
        raise last_err
    pred = np.zeros((B, HW), np.float32)
    xmin = np.zeros((B, HW), np.float32)
    for b in range(B):
        for h in range(2):
            ph = np.asarray(res[2 * b + h]["pred"]).reshape(32, W)
            xh = np.asarray(res[2 * b + h]["xminv"]).reshape(32, W)
            if h == 1:
                ph, xh = ph[::-1, :], xh[::-1, :]
            rows = slice(0, 32) if h == 0 else slice(32, 64)
            pred[b].reshape(H, W)[rows] = ph
            xmin[b].reshape(H, W)[rows] = xh
    return pred, xmin
